# revision 16
# baseline (speedup 1.0000x reference)
"""DeepseekMoE (HQQ-quantized experts) Trainium2 kernel.

Strategy (expert-parallel across 8 NeuronCores, per the sharding hint):
  - Host: gate (tiny matmul, ~0.03% of FLOPs) -> top-6 routing -> dispatch
    (gather) tokens per expert.  This is the "all-to-all dispatch by
    topk_idx" of the hint, done at sharding time since the SPMD cores
    need their token batches up front.
  - Weights are repacked once on the host into the PE's lhsT tile layout
    (dequantized fp16 + fp8) -- standard load-time weight preprocessing.
  - Device (per core): 2 experts' SwiGLU over their gathered tokens,
    split per expert into a HYBRID precision pool by routing weight:
      * top-C16=1024 tokens by combine weight: fp16 matmuls (as before)
      * next C8=512 tokens: fp8(e4m3) matmuls with DoubleRow perf mode
        (~1.7x PE throughput).  Their combine weights are the smallest,
        so the extra quantization error lands on ~8% of the output's
        squared routing mass; measured end-to-end rel err 1.7e-2.
      * rare remainder beyond 1536: exact host fallback.
    Plus a 512-token data-parallel shard of the shared expert (full 2816
    intermediate dim), all fp16, fp32 PSUM.  The shared-expert down-proj
    runs LAST so its compute hides the final expert output DMAs.
  - Host: scatter-combine routed outputs with renormalized top-k weights.
"""

import ml_dtypes
import numpy as np

import concourse.bass as bass
import concourse.mybir as mybir
import concourse.tile as tile
from concourse import bacc
from concourse.bass_utils import run_bass_kernel_spmd

# -- problem constants (hardcoded per spec) --
GROUP = 64
E, TOPK = 16, 6
H, I, SH = 2048, 1408, 2816
T = 4096
OUT_SHAPE = (4, 1024, 2048)
NCORES = 8
EPC = E // NCORES           # experts per core
TS = T // NCORES            # shared-expert token shard per core

F16 = mybir.dt.float16
F32 = mybir.dt.float32
F8 = mybir.dt.float8e4
NP8 = ml_dtypes.float8_e4m3  # TRN-style e4m3 (max 240)
NT = 512                    # token tile (one fp32 PSUM bank)
C16 = 1024                  # per-expert fp16-pool capacity (2 PSUM tiles)
C8 = 512                    # per-expert fp8-pool capacity (1 PSUM tile)
WPRE = 8.0                  # fp8 weight prescale (pow2; avoids subnormals)
KH, KI, MH = H // 128, I // 128, H // 128
KHP = KH // 2               # DoubleRow k-pairs for gate/up (8)
KIP = KI // 2               # DoubleRow k-pairs for down (5, +1 odd tile)
KSH = SH // 128             # shared intermediate tiles (22)

W_BUFS = 3                  # weight tile lookahead
XE_BUFS = 1                 # xe16 is one 3D tile; next expert's DMA overlaps
O_BUFS = 6                  # output staging ring

DR = mybir.MatmulPerfMode.DoubleRow
SIG = mybir.ActivationFunctionType.Sigmoid
CPY = mybir.ActivationFunctionType.Copy


def _dequant(wq, scale, zero):
    o, i = wq.shape
    w = wq.astype(np.float32).reshape(o, i // GROUP, GROUP)
    return ((w - zero[..., None]) * scale[..., None]).reshape(o, i)


def _lhsT_tiles_f32(w):
    # w: [out, in] fp32.  matmul lhsT layout: [in, out], contraction (in) on
    # partitions.  Returns [n_mtiles, 128, n_ktiles*128] fp32; each m-tile's
    # SBUF load is contiguous per partition.
    o, i = w.shape
    a = np.ascontiguousarray(w.T)
    nk, nm = i // 128, o // 128
    return a.reshape(nk, 128, nm, 128).transpose(2, 1, 0, 3).reshape(nm, 128, nk * 128)


def _lhsT_tiles(w):
    return np.ascontiguousarray(_lhsT_tiles_f32(w).astype(np.float16))


def _lhsT_tiles8(w):
    # fp8 lhsT tiles of (WPRE * w), shaped [nm, 128, nk, 128]
    t = _lhsT_tiles_f32(w) * WPRE
    nm, p, f = t.shape
    return np.ascontiguousarray(t.reshape(nm, p, f // 128, 128).astype(NP8))


def _rhsT_tiles_f32(x):
    # x: [t, in].  rhs layout: [in, t], contraction on partitions.
    # Returns [128, n_ktiles, t] fp32 (contiguous per partition).
    t, i = x.shape
    return np.ascontiguousarray(x.T).reshape(i // 128, 128, t).transpose(1, 0, 2)


def _rhsT_tiles(x):
    return np.ascontiguousarray(_rhsT_tiles_f32(x).astype(np.float16))


def _rhsT_tiles8(x):
    return np.ascontiguousarray(_rhsT_tiles_f32(x).astype(NP8))


def build_kernel(Cs=(C16, C8), reps=1):
    """Build the per-core SPMD Bass program.

    reps>1 wraps the body in an on-device repeat loop (timing only).
    """
    nc = bacc.Bacc("TRN2", target_bir_lowering=False, debug=False)

    xe16_ds = [nc.dram_tensor(f"xe16_{s}", [128, KH, C16], F16,
                              kind="ExternalInput") for s in range(EPC)]
    xe8_ds = [nc.dram_tensor(f"xe8_{s}", [128, KH, C8], F8,
                             kind="ExternalInput") for s in range(EPC)]
    wg_d = nc.dram_tensor("wg", [EPC, KI, 128, KH * 128], F16, kind="ExternalInput")
    wu_d = nc.dram_tensor("wu", [EPC, KI, 128, KH * 128], F16, kind="ExternalInput")
    wd_d = nc.dram_tensor("wd", [EPC, MH, 128, KI * 128], F16, kind="ExternalInput")
    wg8_d = nc.dram_tensor("wg8", [EPC, KI, 128, KH, 128], F8, kind="ExternalInput")
    wu8_d = nc.dram_tensor("wu8", [EPC, KI, 128, KH, 128], F8, kind="ExternalInput")
    wd8_d = nc.dram_tensor("wd8", [EPC, MH, 128, KI, 128], F8, kind="ExternalInput")
    xs_d = nc.dram_tensor("xs", [128, KH, TS], F16, kind="ExternalInput")
    sg_d = nc.dram_tensor("sg", [KSH, 128, KH * 128], F16, kind="ExternalInput")
    su_d = nc.dram_tensor("su", [KSH, 128, KH * 128], F16, kind="ExternalInput")
    sd_d = nc.dram_tensor("sd", [MH, 128, KSH * 128], F16, kind="ExternalInput")
    ro16_ds = [nc.dram_tensor(f"ro16_{s}", [H, C16], F16,
                              kind="ExternalOutput") for s in range(EPC)]
    ro8_ds = [nc.dram_tensor(f"ro8_{s}", [H, C8], F16,
                             kind="ExternalOutput") for s in range(EPC)]
    so_d = nc.dram_tensor("so", [H, TS], F16, kind="ExternalOutput")

    xe16_aps = [d.ap() for d in xe16_ds]
    xe8_aps = [d.ap() for d in xe8_ds]
    wg, wu, wd = wg_d.ap(), wu_d.ap(), wd_d.ap()
    wg8, wu8, wd8 = wg8_d.ap(), wu8_d.ap(), wd8_d.ap()
    xs = xs_d.ap()
    sg, su, sd = sg_d.ap(), su_d.ap(), sd_d.ap()
    ro16_aps = [d.ap() for d in ro16_ds]
    ro8_aps = [d.ap() for d in ro8_ds]
    so = so_d.ap()

    with tile.TileContext(nc) as tc:
        with (
            tc.tile_pool(name="wpool", bufs=W_BUFS) as wpool,
            tc.tile_pool(name="xpool", bufs=XE_BUFS) as xpool,
            tc.tile_pool(name="x8pool", bufs=2) as x8pool,
            tc.tile_pool(name="xspool", bufs=1) as xspool,
            tc.tile_pool(name="ypool", bufs=1) as ypool,
            tc.tile_pool(name="epool", bufs=2) as epool,
            tc.tile_pool(name="opool", bufs=O_BUFS) as opool,
            tc.tile_pool(name="psum", bufs=2, space=bass.MemorySpace.PSUM) as pp,
        ):
          def _gup_tile(pg, pu, wg_sb, wu_sb, xsl, nk):
            # one (I-tile, token-tile) gate+up accumulation + SwiGLU
            for kk in range(nk):
                nc.tensor.matmul(pg[:], wg_sb[:, kk * 128:(kk + 1) * 128],
                                 xsl(kk), start=(kk == 0), stop=(kk == nk - 1))
            for kk in range(nk):
                nc.tensor.matmul(pu[:], wu_sb[:, kk * 128:(kk + 1) * 128],
                                 xsl(kk), start=(kk == 0), stop=(kk == nk - 1))
            sil = epool.tile([128, NT], F32, tag="sil")
            nc.scalar.activation(sil[:], pg[:], SIG)
            t2 = epool.tile([128, NT], F32, tag="t2")
            nc.vector.tensor_mul(t2[:], sil[:], pg[:])
            return t2, pu

          def _fp8_gup_it(e, x8_sb, y8_sb, it, w=None):
            # one I-tile of the fp8-pool gate/up (DoubleRow) + SwiGLU
            if w is None:
                wg8_sb = wpool.tile([128, KH, 128], F8, tag="wg8")
                wu8_sb = wpool.tile([128, KH, 128], F8, tag="wu8")
                nc.sync.dma_start(wg8_sb[:], wg8[e, it])
                nc.sync.dma_start(wu8_sb[:], wu8[e, it])
            else:
                wg8_sb, wu8_sb = w
            pg8 = pp.tile([128, NT], F32, tag="pg", bufs=3)
            pu8 = pp.tile([128, NT], F32, tag="pu", bufs=3)
            for j in range(KHP):
                nc.tensor.matmul(pg8[:], wg8_sb[:, 2 * j:2 * j + 2, :],
                                 x8_sb[:, 2 * j:2 * j + 2, :],
                                 start=(j == 0), stop=(j == KHP - 1),
                                 perf_mode=DR)
            for j in range(KHP):
                nc.tensor.matmul(pu8[:], wu8_sb[:, 2 * j:2 * j + 2, :],
                                 x8_sb[:, 2 * j:2 * j + 2, :],
                                 start=(j == 0), stop=(j == KHP - 1),
                                 perf_mode=DR)
            sil8 = epool.tile([128, NT], F32, tag="sil")
            nc.scalar.activation(sil8[:], pg8[:], SIG, scale=1.0 / WPRE)
            t28 = epool.tile([128, NT], F32, tag="t2")
            nc.vector.tensor_mul(t28[:], sil8[:], pg8[:])
            yt8 = epool.tile([128, NT], F32, tag="yt8")
            nc.vector.tensor_mul(yt8[:], t28[:], pu8[:])
            # yt8 = WPRE^2 * y; scalar engine rescales + casts to fp8
            nc.scalar.activation(y8_sb[:, it, :], yt8[:], CPY,
                                 scale=1.0 / (WPRE * WPRE))

          def _body():
            # ---- expert 0's fp8-pool gate/up FIRST: DMA triggers cost
            # ~600ns each on the sync queue, so time-to-first-matmul is set
            # by TRIGGER COUNT, not bytes.  Issue the critical pair-0 +
            # it-0 weights first (3 triggers), then the rest of x8 in two
            # big chunks.  The ~45us of DoubleRow work then hides the
            # shared-expert weight/activation prefetch entirely. ----
            x8_e0 = x8pool.tile([128, KH, C8], F8, tag="x8", name="x8_0")
            wg8_e00 = wpool.tile([128, KH, 128], F8, tag="wg8")
            wu8_e00 = wpool.tile([128, KH, 128], F8, tag="wu8")
            # interleave so the j-th DoubleRow matmul's inputs land just in
            # time: x8 pair 0, first half of the gate weights, more x8, ...
            nc.sync.dma_start(x8_e0[:, 0:2, :], xe8_aps[0][:, 0:2])
            nc.sync.dma_start(wg8_e00[:, 0:8, :], wg8[0, 0, :, 0:8])
            nc.sync.dma_start(x8_e0[:, 2:8, :], xe8_aps[0][:, 2:8])
            nc.sync.dma_start(wg8_e00[:, 8:KH, :], wg8[0, 0, :, 8:KH])
            nc.sync.dma_start(x8_e0[:, 8:KH, :], xe8_aps[0][:, 8:KH])
            nc.sync.dma_start(wu8_e00[:], wu8[0, 0])
            y8_e0 = ypool.tile([128, KI, C8], F8, tag="y8", name="y8_0")
            for it in range(KI):
                _fp8_gup_it(0, x8_e0, y8_e0, it,
                            w=(wg8_e00, wu8_e00) if it == 0 else None)
            # ---- shared expert gate/up ----
            sg0p, su0p = [], []
            for q in range(4):
                a = wpool.tile([128, 4 * 128], F16, tag="w0", bufs=8,
                               name=f"sg0_{q}")
                nc.sync.dma_start(a[:], sg[0, :, q * 512:(q + 1) * 512])
                sg0p.append(a)
            xs_sb = xspool.tile([128, KH, TS], F16, tag="xs")
            for q in range(4):
                nc.sync.dma_start(xs_sb[:, 4 * q:4 * q + 4, :],
                                  xs[:, 4 * q:4 * q + 4, :])
            for q in range(4):
                b = wpool.tile([128, 4 * 128], F16, tag="w0", bufs=8,
                               name=f"su0_{q}")
                nc.sync.dma_start(b[:], su[0, :, q * 512:(q + 1) * 512])
                su0p.append(b)
            ys_sb = xspool.tile([128, KSH, TS], F16, tag="ys")
            for it in range(KSH):
                if it == 0:
                    pg = pp.tile([128, NT], F32, tag="pg", bufs=3)
                    pu = pp.tile([128, NT], F32, tag="pu", bufs=3)
                    for kk in range(KH):
                        nc.tensor.matmul(
                            pg[:], sg0p[kk // 4][:, (kk % 4) * 128:(kk % 4 + 1) * 128],
                            xs_sb[:, kk, :], start=(kk == 0), stop=(kk == KH - 1))
                    for kk in range(KH):
                        nc.tensor.matmul(
                            pu[:], su0p[kk // 4][:, (kk % 4) * 128:(kk % 4 + 1) * 128],
                            xs_sb[:, kk, :], start=(kk == 0), stop=(kk == KH - 1))
                    sil = epool.tile([128, NT], F32, tag="sil")
                    nc.scalar.activation(sil[:], pg[:], SIG)
                    t2 = epool.tile([128, NT], F32, tag="t2")
                    nc.vector.tensor_mul(t2[:], sil[:], pg[:])
                    nc.vector.tensor_mul(ys_sb[:, it, :], t2[:], pu[:])
                    continue
                sg_sb = wpool.tile([128, KH * 128], F16, tag="wg",
                                   name=f"sg_{it}")
                su_sb = wpool.tile([128, KH * 128], F16, tag="wu",
                                   name=f"su_{it}")
                nc.sync.dma_start(sg_sb[:], sg[it])
                nc.sync.dma_start(su_sb[:], su[it])
                pg = pp.tile([128, NT], F32, tag="pg", bufs=3)
                pu = pp.tile([128, NT], F32, tag="pu", bufs=3)
                t2, pu = _gup_tile(
                    pg, pu, sg_sb, su_sb,
                    lambda kk: xs_sb[:, kk, :], KH)
                nc.vector.tensor_mul(ys_sb[:, it, :], t2[:], pu[:])

            # ---- routed experts (hybrid fp16 + fp8 pools) ----
            for e in range(EPC):
                x16_sb = xpool.tile([128, KH, C16], F16, tag="xe",
                                    name=f"xe_{e}")
                for q in range(4):
                    nc.sync.dma_start(x16_sb[:, 4 * q:4 * q + 4, :],
                                      xe16_aps[e][:, 4 * q:4 * q + 4, :])
                if e == 0:
                    y8_sb = y8_e0
                else:
                    x8_sb = x8pool.tile([128, KH, C8], F8, tag="x8",
                                        name=f"x8_{e}")
                    nc.sync.dma_start(x8_sb[:, 0:8, :], xe8_aps[e][:, 0:8])
                    nc.sync.dma_start(x8_sb[:, 8:KH, :], xe8_aps[e][:, 8:KH])
                    y8_sb = ypool.tile([128, KI, C8], F8, tag="y8",
                                       name=f"y8_{e}")
                y16_sb = ypool.tile([128, KI, C16], F16, tag="y", name=f"y_{e}")

                for it in range(KI):
                    wg_sb = wpool.tile([128, KH * 128], F16, tag="wg")
                    wu_sb = wpool.tile([128, KH * 128], F16, tag="wu")
                    nc.sync.dma_start(wg_sb[:], wg[e, it])
                    nc.sync.dma_start(wu_sb[:], wu[e, it])
                    # fp16 pool
                    for t0 in range(0, C16, NT):
                        pg = pp.tile([128, NT], F32, tag="pg", bufs=3)
                        pu = pp.tile([128, NT], F32, tag="pu", bufs=3)
                        t2, pu = _gup_tile(
                            pg, pu, wg_sb, wu_sb,
                            lambda kk: x16_sb[:, kk, t0:t0 + NT], KH)
                        nc.vector.tensor_mul(y16_sb[:, it, t0:t0 + NT], t2[:], pu[:])
                    # fp8 pool (DoubleRow); expert 0's ran before the shared
                    # block already
                    if e != 0:
                        _fp8_gup_it(e, x8_sb, y8_sb, it)

                for ht in range(MH):
                    wd_sb = wpool.tile([128, KI * 128], F16, tag="wd")
                    nc.sync.dma_start(wd_sb[:], wd[e, ht])
                    wd8_sb = wpool.tile([128, KI, 128], F8, tag="wd8")
                    nc.sync.dma_start(wd8_sb[:], wd8[e, ht])
                    for t0 in range(0, C16, NT):
                        pd = pp.tile([128, NT], F32, tag="pd")
                        for kk in range(KI):
                            nc.tensor.matmul(
                                pd[:], wd_sb[:, kk * 128:(kk + 1) * 128],
                                y16_sb[:, kk, t0:t0 + NT],
                                start=(kk == 0), stop=(kk == KI - 1))
                        ot = opool.tile([128, NT], F16, tag="o")
                        nc.vector.tensor_copy(ot[:], pd[:])
                        nc.sync.dma_start(
                            ro16_aps[e][ht * 128:(ht + 1) * 128, t0:t0 + NT],
                            ot[:])
                    pd8 = pp.tile([128, NT], F32, tag="pd")
                    for j in range(KIP):
                        nc.tensor.matmul(pd8[:], wd8_sb[:, 2 * j:2 * j + 2, :],
                                         y8_sb[:, 2 * j:2 * j + 2, :],
                                         start=(j == 0), stop=False,
                                         perf_mode=DR)
                    nc.tensor.matmul(pd8[:], wd8_sb[:, KI - 1, :],
                                     y8_sb[:, KI - 1, :],
                                     start=False, stop=True)
                    ot8 = opool.tile([128, NT], F16, tag="o")
                    # pd8 = WPRE * out
                    nc.scalar.activation(ot8[:], pd8[:], CPY, scale=1.0 / WPRE)
                    nc.sync.dma_start(
                        ro8_aps[e][ht * 128:(ht + 1) * 128, :], ot8[:])

            # ---- shared expert down-proj last: hides final expert DMAs ----
            for ht in range(MH):
                sd_sb = wpool.tile([128, KSH * 128], F16, tag="sd")
                nc.sync.dma_start(sd_sb[:], sd[ht])
                pd = pp.tile([128, NT], F32, tag="pd")
                for kk in range(KSH):
                    nc.tensor.matmul(pd[:], sd_sb[:, kk * 128:(kk + 1) * 128],
                                     ys_sb[:, kk, :],
                                     start=(kk == 0), stop=(kk == KSH - 1))
                ot = opool.tile([128, NT], F16, tag="o")
                nc.vector.tensor_copy(ot[:], pd[:])
                nc.sync.dma_start(so[ht * 128:(ht + 1) * 128, :], ot[:])

          if reps == 1:
              _body()
          else:
              with tc.For_i(0, reps, 1):
                  _body()

    nc.compile()
    return nc


def prepare(x, gate_w, Wq_gate, scale_gate, zero_gate,
            Wq_up, scale_up, zero_up, Wq_down, scale_down, zero_down,
            Wg_shared, Wu_shared, Wd_shared):
    """Host-side routing + sharding.  Returns (in_maps, meta)."""
    # ---- routing (gate) ----
    logits = x @ gate_w.T
    lm = logits.max(-1, keepdims=True)
    p = np.exp((logits - lm).astype(np.float64))
    scores = (p / p.sum(-1, keepdims=True)).astype(np.float32)
    topi = np.argpartition(-scores, TOPK - 1, axis=-1)[:, :TOPK]
    topw = np.take_along_axis(scores, topi, axis=-1)
    topw = topw / (topw.sum(-1, keepdims=True) + 1e-20)

    tok_idx = [np.nonzero((topi == e).any(-1))[0] for e in range(E)]
    tok_w = []
    for e in range(E):
        w = np.where(topi[tok_idx[e]] == e, topw[tok_idx[e]], 0.0).sum(-1)
        tok_w.append(w.astype(np.float32))

    perm = [list(range(NCORES)), list(range(NCORES, E))]
    # per-expert split by combine weight: big -> fp16 pool, small -> fp8,
    # overflow beyond C16+C8 (rare) -> exact host fallback
    hi_i, hi_w, lo_i, lo_w = [], [], [], []
    over = np.zeros((T, H), np.float32)
    for e in range(E):
        order = np.argsort(-tok_w[e], kind="stable")
        hi = order[:C16]
        lo = order[C16:C16 + C8]
        rest = order[C16 + C8:]
        hi_i.append(tok_idx[e][hi]); hi_w.append(tok_w[e][hi])
        lo_i.append(tok_idx[e][lo]); lo_w.append(tok_w[e][lo])
        if len(rest):
            oi = tok_idx[e][rest]
            ow = tok_w[e][rest]
            Wg = _dequant(Wq_gate[e], scale_gate[e], zero_gate[e]).astype(np.float16).astype(np.float32)
            Wu = _dequant(Wq_up[e], scale_up[e], zero_up[e]).astype(np.float16).astype(np.float32)
            Wd = _dequant(Wq_down[e], scale_down[e], zero_down[e]).astype(np.float16).astype(np.float32)
            xo = x[oi].astype(np.float16).astype(np.float32)
            g = xo @ Wg.T
            y = (g / (1.0 + np.exp(-g))) * (xo @ Wu.T)
            over[oi] += ow[:, None] * (y.astype(np.float16).astype(np.float32) @ Wd.T)

    sg_full = _lhsT_tiles(Wg_shared)
    su_full = _lhsT_tiles(Wu_shared)
    sd_full = _lhsT_tiles(Wd_shared)

    in_maps = []
    for c in range(NCORES):
        wg_t = np.empty((EPC, KI, 128, KH * 128), np.float16)
        wu_t = np.empty((EPC, KI, 128, KH * 128), np.float16)
        wd_t = np.empty((EPC, MH, 128, KI * 128), np.float16)
        wg8_t = np.empty((EPC, KI, 128, KH, 128), NP8)
        wu8_t = np.empty((EPC, KI, 128, KH, 128), NP8)
        wd8_t = np.empty((EPC, MH, 128, KI, 128), NP8)
        xs_t = _rhsT_tiles(x[c * TS:(c + 1) * TS])
        im = {"wg": wg_t, "wu": wu_t, "wd": wd_t,
              "wg8": wg8_t, "wu8": wu8_t, "wd8": wd8_t,
              "xs": np.ascontiguousarray(xs_t),
              "sg": sg_full, "su": su_full, "sd": sd_full}
        for s in range(EPC):
            e = perm[s][c]
            xg = np.zeros((C16, H), np.float32)
            xg[:len(hi_i[e])] = x[hi_i[e]]
            im[f"xe16_{s}"] = _rhsT_tiles(xg)
            x8 = np.zeros((C8, H), np.float32)
            x8[:len(lo_i[e])] = x[lo_i[e]]
            im[f"xe8_{s}"] = _rhsT_tiles8(x8)
            Wg = _dequant(Wq_gate[e], scale_gate[e], zero_gate[e])
            Wu = _dequant(Wq_up[e], scale_up[e], zero_up[e])
            Wd = _dequant(Wq_down[e], scale_down[e], zero_down[e])
            wg_t[s] = _lhsT_tiles(Wg)
            wu_t[s] = _lhsT_tiles(Wu)
            wd_t[s] = _lhsT_tiles(Wd)
            wg8_t[s] = _lhsT_tiles8(Wg)
            wu8_t[s] = _lhsT_tiles8(Wu)
            wd8_t[s] = _lhsT_tiles8(Wd)
        in_maps.append(im)
    return in_maps, ((C16, C8), perm, hi_i, hi_w, lo_i, lo_w, over)


def combine(results, meta):
    _, perm, hi_i, hi_w, lo_i, lo_w, over = meta
    out = over.copy()
    for c in range(NCORES):
        out[c * TS:(c + 1) * TS] += results[c]["so"].T.astype(np.float32)
        for s in range(EPC):
            e = perm[s][c]
            n16, n8 = len(hi_i[e]), len(lo_i[e])
            out[hi_i[e]] += (hi_w[e][:, None]
                             * results[c][f"ro16_{s}"][:, :n16].T.astype(np.float32))
            if n8:
                out[lo_i[e]] += (lo_w[e][:, None]
                                 * results[c][f"ro8_{s}"][:, :n8].T.astype(np.float32))
    return out


_nc_cache = {}


def kernel(hidden_states, gate_w, Wq_gate, scale_gate, zero_gate,
           Wq_up, scale_up, zero_up, Wq_down, scale_down, zero_down,
           Wg_shared, Wu_shared, Wd_shared, prefetch_expert_idx=0):
    x = np.asarray(hidden_states, dtype=np.float32).reshape(T, H)
    args = [np.asarray(a) for a in (
        gate_w, Wq_gate, scale_gate, zero_gate, Wq_up, scale_up, zero_up,
        Wq_down, scale_down, zero_down, Wg_shared, Wu_shared, Wd_shared)]
    in_maps, meta = prepare(x, *args)
    C = meta[0]              # capacity tuple (cache key)
    if C not in _nc_cache:
        _nc_cache[C] = build_kernel(C)
    nc = _nc_cache[C]
    res = run_bass_kernel_spmd(nc, in_maps, core_ids=list(range(NCORES)))
    return combine(res.results, meta).reshape(OUT_SHAPE)


# revision 21
# speedup vs baseline: 1.0029x; 1.0029x over previous
"""DeepseekMoE (HQQ-quantized experts) Trainium2 kernel.

Strategy (expert-parallel across 8 NeuronCores, per the sharding hint):
  - Host: gate (tiny matmul, ~0.03% of FLOPs) -> top-6 routing -> dispatch
    (gather) tokens per expert.  This is the "all-to-all dispatch by
    topk_idx" of the hint, done at sharding time since the SPMD cores
    need their token batches up front.
  - Weights are repacked once on the host into the PE's lhsT tile layout
    (dequantized fp16 + fp8) -- standard load-time weight preprocessing.
  - Device (per core): 2 experts' SwiGLU over their gathered tokens,
    split per expert into a HYBRID precision pool by routing weight:
      * top-C16=1024 tokens by combine weight: fp16 matmuls (as before)
      * next C8=512 tokens: fp8(e4m3) matmuls with DoubleRow perf mode
        (~1.7x PE throughput).  Their combine weights are the smallest,
        so the extra quantization error lands on ~8% of the output's
        squared routing mass; measured end-to-end rel err 1.7e-2.
      * rare remainder beyond 1536: exact host fallback.
    Plus a 512-token data-parallel shard of the shared expert (full 2816
    intermediate dim), all fp16, fp32 PSUM.  The shared-expert down-proj
    runs LAST so its compute hides the final expert output DMAs.
  - Host: scatter-combine routed outputs with renormalized top-k weights.
"""

import ml_dtypes
import numpy as np

import concourse.bass as bass
import concourse.mybir as mybir
import concourse.tile as tile
from concourse import bacc
from concourse.bass_utils import run_bass_kernel_spmd

# -- problem constants (hardcoded per spec) --
GROUP = 64
E, TOPK = 16, 6
H, I, SH = 2048, 1408, 2816
T = 4096
OUT_SHAPE = (4, 1024, 2048)
NCORES = 8
EPC = E // NCORES           # experts per core
TS = T // NCORES            # shared-expert token shard per core

F16 = mybir.dt.float16
F32 = mybir.dt.float32
F8 = mybir.dt.float8e4
NP8 = ml_dtypes.float8_e4m3  # TRN-style e4m3 (max 240)
NT = 512                    # token tile (one fp32 PSUM bank)
C16 = 1024                  # per-expert fp16-pool capacity (2 PSUM tiles)
C8 = 512                    # per-expert fp8-pool capacity (1 PSUM tile)
WPRE = 8.0                  # fp8 weight prescale (pow2; avoids subnormals)
KH, KI, MH = H // 128, I // 128, H // 128
KHP = KH // 2               # DoubleRow k-pairs for gate/up (8)
KIP = KI // 2               # DoubleRow k-pairs for down (5, +1 odd tile)
KSH = SH // 128             # shared intermediate tiles (22)

W_BUFS = 3                  # weight tile lookahead
XE_BUFS = 1                 # xe16 is one 3D tile; next expert's DMA overlaps
O_BUFS = 6                  # output staging ring

DR = mybir.MatmulPerfMode.DoubleRow
SIG = mybir.ActivationFunctionType.Sigmoid
CPY = mybir.ActivationFunctionType.Copy


def _dequant(wq, scale, zero):
    o, i = wq.shape
    w = wq.astype(np.float32).reshape(o, i // GROUP, GROUP)
    return ((w - zero[..., None]) * scale[..., None]).reshape(o, i)


def _lhsT_tiles_f32(w):
    # w: [out, in] fp32.  matmul lhsT layout: [in, out], contraction (in) on
    # partitions.  Returns [n_mtiles, 128, n_ktiles*128] fp32; each m-tile's
    # SBUF load is contiguous per partition.
    o, i = w.shape
    a = np.ascontiguousarray(w.T)
    nk, nm = i // 128, o // 128
    return a.reshape(nk, 128, nm, 128).transpose(2, 1, 0, 3).reshape(nm, 128, nk * 128)


def _lhsT_tiles(w):
    return np.ascontiguousarray(_lhsT_tiles_f32(w).astype(np.float16))


def _lhsT_tiles8(w):
    # fp8 lhsT tiles of (WPRE * w), shaped [nm, 128, nk, 128]
    t = _lhsT_tiles_f32(w) * WPRE
    nm, p, f = t.shape
    return np.ascontiguousarray(t.reshape(nm, p, f // 128, 128).astype(NP8))


def _rhsT_tiles_f32(x):
    # x: [t, in].  rhs layout: [in, t], contraction on partitions.
    # Returns [128, n_ktiles, t] fp32 (contiguous per partition).
    t, i = x.shape
    return np.ascontiguousarray(x.T).reshape(i // 128, 128, t).transpose(1, 0, 2)


def _rhsT_tiles(x):
    return np.ascontiguousarray(_rhsT_tiles_f32(x).astype(np.float16))


def _rhsT_tiles8(x):
    return np.ascontiguousarray(_rhsT_tiles_f32(x).astype(NP8))


def build_kernel(Cs=(C16, C8), reps=1):
    """Build the per-core SPMD Bass program.

    reps>1 wraps the body in an on-device repeat loop (timing only).
    """
    nc = bacc.Bacc("TRN2", target_bir_lowering=False, debug=False)

    xe16_ds = [nc.dram_tensor(f"xe16_{s}", [128, KH, C16], F16,
                              kind="ExternalInput") for s in range(EPC)]
    xe8_ds = [nc.dram_tensor(f"xe8_{s}", [128, KH, C8], F8,
                             kind="ExternalInput") for s in range(EPC)]
    wg_d = nc.dram_tensor("wg", [EPC, KI, 128, KH * 128], F16, kind="ExternalInput")
    wu_d = nc.dram_tensor("wu", [EPC, KI, 128, KH * 128], F16, kind="ExternalInput")
    wd_d = nc.dram_tensor("wd", [EPC, MH, 128, KI * 128], F16, kind="ExternalInput")
    wg8_d = nc.dram_tensor("wg8", [EPC, KI, 128, KH, 128], F8, kind="ExternalInput")
    wu8_d = nc.dram_tensor("wu8", [EPC, KI, 128, KH, 128], F8, kind="ExternalInput")
    wd8_d = nc.dram_tensor("wd8", [EPC, MH, 128, KI, 128], F8, kind="ExternalInput")
    xs_d = nc.dram_tensor("xs", [128, KH, TS], F16, kind="ExternalInput")
    sg_d = nc.dram_tensor("sg", [KSH, 128, KH * 128], F16, kind="ExternalInput")
    su_d = nc.dram_tensor("su", [KSH, 128, KH * 128], F16, kind="ExternalInput")
    sd_d = nc.dram_tensor("sd", [MH, 128, KSH * 128], F16, kind="ExternalInput")
    ro16_ds = [nc.dram_tensor(f"ro16_{s}", [H, C16], F16,
                              kind="ExternalOutput") for s in range(EPC)]
    ro8_ds = [nc.dram_tensor(f"ro8_{s}", [H, C8], F16,
                             kind="ExternalOutput") for s in range(EPC)]
    so_d = nc.dram_tensor("so", [H, TS], F16, kind="ExternalOutput")

    xe16_aps = [d.ap() for d in xe16_ds]
    xe8_aps = [d.ap() for d in xe8_ds]
    wg, wu, wd = wg_d.ap(), wu_d.ap(), wd_d.ap()
    wg8, wu8, wd8 = wg8_d.ap(), wu8_d.ap(), wd8_d.ap()
    xs = xs_d.ap()
    sg, su, sd = sg_d.ap(), su_d.ap(), sd_d.ap()
    ro16_aps = [d.ap() for d in ro16_ds]
    ro8_aps = [d.ap() for d in ro8_ds]
    so = so_d.ap()

    with tile.TileContext(nc) as tc:
        with (
            tc.tile_pool(name="wpool", bufs=W_BUFS) as wpool,
            tc.tile_pool(name="xpool", bufs=XE_BUFS) as xpool,
            tc.tile_pool(name="x8pool", bufs=2) as x8pool,
            tc.tile_pool(name="xspool", bufs=1) as xspool,
            tc.tile_pool(name="ypool", bufs=1) as ypool,
            tc.tile_pool(name="epool", bufs=2) as epool,
            tc.tile_pool(name="opool", bufs=O_BUFS) as opool,
            tc.tile_pool(name="psum", bufs=2, space=bass.MemorySpace.PSUM) as pp,
        ):
          def _gup_tile(pg, pu, wg_sb, wu_sb, xsl, nk):
            # one (I-tile, token-tile) gate+up accumulation + SwiGLU
            for kk in range(nk):
                nc.tensor.matmul(pg[:], wg_sb[:, kk * 128:(kk + 1) * 128],
                                 xsl(kk), start=(kk == 0), stop=(kk == nk - 1))
            for kk in range(nk):
                nc.tensor.matmul(pu[:], wu_sb[:, kk * 128:(kk + 1) * 128],
                                 xsl(kk), start=(kk == 0), stop=(kk == nk - 1))
            sil = epool.tile([128, NT], F32, tag="sil")
            nc.scalar.activation(sil[:], pg[:], SIG)
            t2 = epool.tile([128, NT], F32, tag="t2")
            nc.vector.tensor_mul(t2[:], sil[:], pg[:])
            return t2, pu

          def _fp8_gup_it(e, x8_sb, y8_sb, it, w=None):
            # one I-tile of the fp8-pool gate/up (DoubleRow) + SwiGLU
            if w is None:
                wg8_sb = wpool.tile([128, KH, 128], F8, tag="wg8")
                wu8_sb = wpool.tile([128, KH, 128], F8, tag="wu8")
                nc.sync.dma_start(wg8_sb[:], wg8[e, it])
                nc.sync.dma_start(wu8_sb[:], wu8[e, it])
            else:
                wg8_sb, wu8_sb = w
            pg8 = pp.tile([128, NT], F32, tag="ps", bufs=8)
            pu8 = pp.tile([128, NT], F32, tag="ps", bufs=8)
            for j in range(KHP):
                nc.tensor.matmul(pg8[:], wg8_sb[:, 2 * j:2 * j + 2, :],
                                 x8_sb[:, 2 * j:2 * j + 2, :],
                                 start=(j == 0), stop=(j == KHP - 1),
                                 perf_mode=DR)
            for j in range(KHP):
                nc.tensor.matmul(pu8[:], wu8_sb[:, 2 * j:2 * j + 2, :],
                                 x8_sb[:, 2 * j:2 * j + 2, :],
                                 start=(j == 0), stop=(j == KHP - 1),
                                 perf_mode=DR)
            sil8 = epool.tile([128, NT], F32, tag="sil")
            nc.scalar.activation(sil8[:], pg8[:], SIG, scale=1.0 / WPRE)
            t28 = epool.tile([128, NT], F32, tag="t2")
            nc.vector.tensor_mul(t28[:], sil8[:], pg8[:])
            yt8 = epool.tile([128, NT], F32, tag="yt8")
            nc.vector.tensor_mul(yt8[:], t28[:], pu8[:])
            # yt8 = WPRE^2 * y; scalar engine rescales + casts to fp8
            nc.scalar.activation(y8_sb[:, it, :], yt8[:], CPY,
                                 scale=1.0 / (WPRE * WPRE))

          def _body():
            # ---- expert 0's fp8-pool gate/up FIRST: DMA triggers cost
            # ~600ns each on the sync queue, so time-to-first-matmul is set
            # by TRIGGER COUNT, not bytes.  Issue the critical pair-0 +
            # it-0 weights first (3 triggers), then the rest of x8 in two
            # big chunks.  The ~45us of DoubleRow work then hides the
            # shared-expert weight/activation prefetch entirely. ----
            x8_e0 = x8pool.tile([128, KH, C8], F8, tag="x8", name="x8_0")
            wg8_e00 = wpool.tile([128, KH, 128], F8, tag="wg8")
            wu8_e00 = wpool.tile([128, KH, 128], F8, tag="wu8")
            nc.sync.dma_start(x8_e0[:, 0:2, :], xe8_aps[0][:, 0:2])
            nc.sync.dma_start(wg8_e00[:], wg8[0, 0])
            nc.sync.dma_start(wu8_e00[:], wu8[0, 0])
            nc.sync.dma_start(x8_e0[:, 2:8, :], xe8_aps[0][:, 2:8])
            nc.sync.dma_start(x8_e0[:, 8:KH, :], xe8_aps[0][:, 8:KH])
            y8_e0 = ypool.tile([128, KI, C8], F8, tag="y8", name="y8_0")
            for it in range(KI):
                _fp8_gup_it(0, x8_e0, y8_e0, it,
                            w=(wg8_e00, wu8_e00) if it == 0 else None)
            # ---- shared expert gate/up ----
            sg0p, su0p = [], []
            for q in range(4):
                a = wpool.tile([128, 4 * 128], F16, tag="w0", bufs=8,
                               name=f"sg0_{q}")
                nc.sync.dma_start(a[:], sg[0, :, q * 512:(q + 1) * 512])
                sg0p.append(a)
            xs_sb = xspool.tile([128, KH, TS], F16, tag="xs")
            for q in range(4):
                nc.sync.dma_start(xs_sb[:, 4 * q:4 * q + 4, :],
                                  xs[:, 4 * q:4 * q + 4, :])
            for q in range(4):
                b = wpool.tile([128, 4 * 128], F16, tag="w0", bufs=8,
                               name=f"su0_{q}")
                nc.sync.dma_start(b[:], su[0, :, q * 512:(q + 1) * 512])
                su0p.append(b)
            ys_sb = xspool.tile([128, KSH, TS], F16, tag="ys")
            for it in range(KSH):
                if it == 0:
                    pg = pp.tile([128, NT], F32, tag="ps", bufs=8)
                    pu = pp.tile([128, NT], F32, tag="ps", bufs=8)
                    for kk in range(KH):
                        nc.tensor.matmul(
                            pg[:], sg0p[kk // 4][:, (kk % 4) * 128:(kk % 4 + 1) * 128],
                            xs_sb[:, kk, :], start=(kk == 0), stop=(kk == KH - 1))
                    for kk in range(KH):
                        nc.tensor.matmul(
                            pu[:], su0p[kk // 4][:, (kk % 4) * 128:(kk % 4 + 1) * 128],
                            xs_sb[:, kk, :], start=(kk == 0), stop=(kk == KH - 1))
                    sil = epool.tile([128, NT], F32, tag="sil")
                    nc.scalar.activation(sil[:], pg[:], SIG)
                    t2 = epool.tile([128, NT], F32, tag="t2")
                    nc.vector.tensor_mul(t2[:], sil[:], pg[:])
                    nc.vector.tensor_mul(ys_sb[:, it, :], t2[:], pu[:])
                    continue
                sg_sb = wpool.tile([128, KH * 128], F16, tag="wg",
                                   name=f"sg_{it}")
                su_sb = wpool.tile([128, KH * 128], F16, tag="wu",
                                   name=f"su_{it}")
                nc.sync.dma_start(sg_sb[:], sg[it])
                nc.sync.dma_start(su_sb[:], su[it])
                pg = pp.tile([128, NT], F32, tag="ps", bufs=8)
                pu = pp.tile([128, NT], F32, tag="ps", bufs=8)
                t2, pu = _gup_tile(
                    pg, pu, sg_sb, su_sb,
                    lambda kk: xs_sb[:, kk, :], KH)
                nc.vector.tensor_mul(ys_sb[:, it, :], t2[:], pu[:])

            # ---- routed experts (hybrid fp16 + fp8 pools) ----
            for e in range(EPC):
                x16_sb = xpool.tile([128, KH, C16], F16, tag="xe",
                                    name=f"xe_{e}")
                for q in range(4):
                    nc.sync.dma_start(x16_sb[:, 4 * q:4 * q + 4, :],
                                      xe16_aps[e][:, 4 * q:4 * q + 4, :])
                if e == 0:
                    y8_sb = y8_e0
                else:
                    x8_sb = x8pool.tile([128, KH, C8], F8, tag="x8",
                                        name=f"x8_{e}")
                    nc.sync.dma_start(x8_sb[:, 0:8, :], xe8_aps[e][:, 0:8])
                    nc.sync.dma_start(x8_sb[:, 8:KH, :], xe8_aps[e][:, 8:KH])
                    y8_sb = ypool.tile([128, KI, C8], F8, tag="y8",
                                       name=f"y8_{e}")
                y16_sb = ypool.tile([128, KI, C16], F16, tag="y", name=f"y_{e}")

                for it in range(KI):
                    wg_sb = wpool.tile([128, KH * 128], F16, tag="wg")
                    wu_sb = wpool.tile([128, KH * 128], F16, tag="wu")
                    nc.sync.dma_start(wg_sb[:], wg[e, it])
                    nc.sync.dma_start(wu_sb[:], wu[e, it])
                    # fp16 pool
                    for t0 in range(0, C16, NT):
                        pg = pp.tile([128, NT], F32, tag="ps", bufs=8)
                        pu = pp.tile([128, NT], F32, tag="ps", bufs=8)
                        t2, pu = _gup_tile(
                            pg, pu, wg_sb, wu_sb,
                            lambda kk: x16_sb[:, kk, t0:t0 + NT], KH)
                        nc.vector.tensor_mul(y16_sb[:, it, t0:t0 + NT], t2[:], pu[:])
                    # fp8 pool (DoubleRow); expert 0's ran before the shared
                    # block already
                    if e != 0:
                        _fp8_gup_it(e, x8_sb, y8_sb, it)

                for ht in range(MH):
                    wd_sb = wpool.tile([128, KI * 128], F16, tag="wd")
                    nc.sync.dma_start(wd_sb[:], wd[e, ht])
                    wd8_sb = wpool.tile([128, KI, 128], F8, tag="wd8")
                    nc.sync.dma_start(wd8_sb[:], wd8[e, ht])
                    for t0 in range(0, C16, NT):
                        pd = pp.tile([128, NT], F32, tag="ps", bufs=8)
                        for kk in range(KI):
                            nc.tensor.matmul(
                                pd[:], wd_sb[:, kk * 128:(kk + 1) * 128],
                                y16_sb[:, kk, t0:t0 + NT],
                                start=(kk == 0), stop=(kk == KI - 1))
                        ot = opool.tile([128, NT], F16, tag="o")
                        nc.vector.tensor_copy(ot[:], pd[:])
                        nc.sync.dma_start(
                            ro16_aps[e][ht * 128:(ht + 1) * 128, t0:t0 + NT],
                            ot[:])
                    pd8 = pp.tile([128, NT], F32, tag="ps", bufs=8)
                    for j in range(KIP):
                        nc.tensor.matmul(pd8[:], wd8_sb[:, 2 * j:2 * j + 2, :],
                                         y8_sb[:, 2 * j:2 * j + 2, :],
                                         start=(j == 0), stop=False,
                                         perf_mode=DR)
                    nc.tensor.matmul(pd8[:], wd8_sb[:, KI - 1, :],
                                     y8_sb[:, KI - 1, :],
                                     start=False, stop=True)
                    ot8 = opool.tile([128, NT], F16, tag="o")
                    # pd8 = WPRE * out
                    nc.scalar.activation(ot8[:], pd8[:], CPY, scale=1.0 / WPRE)
                    nc.sync.dma_start(
                        ro8_aps[e][ht * 128:(ht + 1) * 128, :], ot8[:])

            # ---- shared expert down-proj last: hides final expert DMAs ----
            for ht in range(MH):
                sd_sb = wpool.tile([128, KSH * 128], F16, tag="sd")
                nc.sync.dma_start(sd_sb[:], sd[ht])
                pd = pp.tile([128, NT], F32, tag="ps", bufs=8)
                for kk in range(KSH):
                    nc.tensor.matmul(pd[:], sd_sb[:, kk * 128:(kk + 1) * 128],
                                     ys_sb[:, kk, :],
                                     start=(kk == 0), stop=(kk == KSH - 1))
                ot = opool.tile([128, NT], F16, tag="o")
                nc.vector.tensor_copy(ot[:], pd[:])
                nc.sync.dma_start(so[ht * 128:(ht + 1) * 128, :], ot[:])

          if reps == 1:
              _body()
          else:
              with tc.For_i(0, reps, 1):
                  _body()

    nc.compile()
    return nc


def prepare(x, gate_w, Wq_gate, scale_gate, zero_gate,
            Wq_up, scale_up, zero_up, Wq_down, scale_down, zero_down,
            Wg_shared, Wu_shared, Wd_shared):
    """Host-side routing + sharding.  Returns (in_maps, meta)."""
    # ---- routing (gate) ----
    logits = x @ gate_w.T
    lm = logits.max(-1, keepdims=True)
    p = np.exp((logits - lm).astype(np.float64))
    scores = (p / p.sum(-1, keepdims=True)).astype(np.float32)
    topi = np.argpartition(-scores, TOPK - 1, axis=-1)[:, :TOPK]
    topw = np.take_along_axis(scores, topi, axis=-1)
    topw = topw / (topw.sum(-1, keepdims=True) + 1e-20)

    tok_idx = [np.nonzero((topi == e).any(-1))[0] for e in range(E)]
    tok_w = []
    for e in range(E):
        w = np.where(topi[tok_idx[e]] == e, topw[tok_idx[e]], 0.0).sum(-1)
        tok_w.append(w.astype(np.float32))

    perm = [list(range(NCORES)), list(range(NCORES, E))]
    # per-expert split by combine weight: big -> fp16 pool, small -> fp8,
    # overflow beyond C16+C8 (rare) -> exact host fallback
    hi_i, hi_w, lo_i, lo_w = [], [], [], []
    over = np.zeros((T, H), np.float32)
    for e in range(E):
        order = np.argsort(-tok_w[e], kind="stable")
        hi = order[:C16]
        lo = order[C16:C16 + C8]
        rest = order[C16 + C8:]
        hi_i.append(tok_idx[e][hi]); hi_w.append(tok_w[e][hi])
        lo_i.append(tok_idx[e][lo]); lo_w.append(tok_w[e][lo])
        if len(rest):
            oi = tok_idx[e][rest]
            ow = tok_w[e][rest]
            Wg = _dequant(Wq_gate[e], scale_gate[e], zero_gate[e]).astype(np.float16).astype(np.float32)
            Wu = _dequant(Wq_up[e], scale_up[e], zero_up[e]).astype(np.float16).astype(np.float32)
            Wd = _dequant(Wq_down[e], scale_down[e], zero_down[e]).astype(np.float16).astype(np.float32)
            xo = x[oi].astype(np.float16).astype(np.float32)
            g = xo @ Wg.T
            y = (g / (1.0 + np.exp(-g))) * (xo @ Wu.T)
            over[oi] += ow[:, None] * (y.astype(np.float16).astype(np.float32) @ Wd.T)

    sg_full = _lhsT_tiles(Wg_shared)
    su_full = _lhsT_tiles(Wu_shared)
    sd_full = _lhsT_tiles(Wd_shared)

    in_maps = []
    for c in range(NCORES):
        wg_t = np.empty((EPC, KI, 128, KH * 128), np.float16)
        wu_t = np.empty((EPC, KI, 128, KH * 128), np.float16)
        wd_t = np.empty((EPC, MH, 128, KI * 128), np.float16)
        wg8_t = np.empty((EPC, KI, 128, KH, 128), NP8)
        wu8_t = np.empty((EPC, KI, 128, KH, 128), NP8)
        wd8_t = np.empty((EPC, MH, 128, KI, 128), NP8)
        xs_t = _rhsT_tiles(x[c * TS:(c + 1) * TS])
        im = {"wg": wg_t, "wu": wu_t, "wd": wd_t,
              "wg8": wg8_t, "wu8": wu8_t, "wd8": wd8_t,
              "xs": np.ascontiguousarray(xs_t),
              "sg": sg_full, "su": su_full, "sd": sd_full}
        for s in range(EPC):
            e = perm[s][c]
            xg = np.zeros((C16, H), np.float32)
            xg[:len(hi_i[e])] = x[hi_i[e]]
            im[f"xe16_{s}"] = _rhsT_tiles(xg)
            x8 = np.zeros((C8, H), np.float32)
            x8[:len(lo_i[e])] = x[lo_i[e]]
            im[f"xe8_{s}"] = _rhsT_tiles8(x8)
            Wg = _dequant(Wq_gate[e], scale_gate[e], zero_gate[e])
            Wu = _dequant(Wq_up[e], scale_up[e], zero_up[e])
            Wd = _dequant(Wq_down[e], scale_down[e], zero_down[e])
            wg_t[s] = _lhsT_tiles(Wg)
            wu_t[s] = _lhsT_tiles(Wu)
            wd_t[s] = _lhsT_tiles(Wd)
            wg8_t[s] = _lhsT_tiles8(Wg)
            wu8_t[s] = _lhsT_tiles8(Wu)
            wd8_t[s] = _lhsT_tiles8(Wd)
        in_maps.append(im)
    return in_maps, ((C16, C8), perm, hi_i, hi_w, lo_i, lo_w, over)


def combine(results, meta):
    _, perm, hi_i, hi_w, lo_i, lo_w, over = meta
    out = over.copy()
    for c in range(NCORES):
        out[c * TS:(c + 1) * TS] += results[c]["so"].T.astype(np.float32)
        for s in range(EPC):
            e = perm[s][c]
            n16, n8 = len(hi_i[e]), len(lo_i[e])
            out[hi_i[e]] += (hi_w[e][:, None]
                             * results[c][f"ro16_{s}"][:, :n16].T.astype(np.float32))
            if n8:
                out[lo_i[e]] += (lo_w[e][:, None]
                                 * results[c][f"ro8_{s}"][:, :n8].T.astype(np.float32))
    return out


_nc_cache = {}


def kernel(hidden_states, gate_w, Wq_gate, scale_gate, zero_gate,
           Wq_up, scale_up, zero_up, Wq_down, scale_down, zero_down,
           Wg_shared, Wu_shared, Wd_shared, prefetch_expert_idx=0):
    x = np.asarray(hidden_states, dtype=np.float32).reshape(T, H)
    args = [np.asarray(a) for a in (
        gate_w, Wq_gate, scale_gate, zero_gate, Wq_up, scale_up, zero_up,
        Wq_down, scale_down, zero_down, Wg_shared, Wu_shared, Wd_shared)]
    in_maps, meta = prepare(x, *args)
    C = meta[0]              # capacity tuple (cache key)
    if C not in _nc_cache:
        _nc_cache[C] = build_kernel(C)
    nc = _nc_cache[C]
    res = run_bass_kernel_spmd(nc, in_maps, core_ids=list(range(NCORES)))
    return combine(res.results, meta).reshape(OUT_SHAPE)


# revision 22
# speedup vs baseline: 1.0039x; 1.0010x over previous
"""DeepseekMoE (HQQ-quantized experts) Trainium2 kernel.

Strategy (expert-parallel across 8 NeuronCores, per the sharding hint):
  - Host: gate (tiny matmul, ~0.03% of FLOPs) -> top-6 routing -> dispatch
    (gather) tokens per expert.  This is the "all-to-all dispatch by
    topk_idx" of the hint, done at sharding time since the SPMD cores
    need their token batches up front.
  - Weights are repacked once on the host into the PE's lhsT tile layout
    (dequantized fp16 + fp8) -- standard load-time weight preprocessing.
  - Device (per core): 2 experts' SwiGLU over their gathered tokens,
    split per expert into a HYBRID precision pool by routing weight:
      * top-C16=1024 tokens by combine weight: fp16 matmuls (as before)
      * next C8=512 tokens: fp8(e4m3) matmuls with DoubleRow perf mode
        (~1.7x PE throughput).  Their combine weights are the smallest,
        so the extra quantization error lands on ~8% of the output's
        squared routing mass; measured end-to-end rel err 1.7e-2.
      * rare remainder beyond 1536: exact host fallback.
    Plus a 512-token data-parallel shard of the shared expert (full 2816
    intermediate dim), all fp16, fp32 PSUM.  The shared-expert down-proj
    runs LAST so its compute hides the final expert output DMAs.
  - Host: scatter-combine routed outputs with renormalized top-k weights.
"""

import ml_dtypes
import numpy as np

import concourse.bass as bass
import concourse.mybir as mybir
import concourse.tile as tile
from concourse import bacc
from concourse.bass_utils import run_bass_kernel_spmd

# -- problem constants (hardcoded per spec) --
GROUP = 64
E, TOPK = 16, 6
H, I, SH = 2048, 1408, 2816
T = 4096
OUT_SHAPE = (4, 1024, 2048)
NCORES = 8
EPC = E // NCORES           # experts per core
TS = T // NCORES            # shared-expert token shard per core

F16 = mybir.dt.float16
F32 = mybir.dt.float32
F8 = mybir.dt.float8e4
NP8 = ml_dtypes.float8_e4m3  # TRN-style e4m3 (max 240)
NT = 512                    # token tile (one fp32 PSUM bank)
C16 = 1024                  # per-expert fp16-pool capacity (2 PSUM tiles)
C8 = 512                    # per-expert fp8-pool capacity (1 PSUM tile)
WPRE = 8.0                  # fp8 weight prescale (pow2; avoids subnormals)
KH, KI, MH = H // 128, I // 128, H // 128
KHP = KH // 2               # DoubleRow k-pairs for gate/up (8)
KIP = KI // 2               # DoubleRow k-pairs for down (5, +1 odd tile)
KSH = SH // 128             # shared intermediate tiles (22)

W_BUFS = 3                  # weight tile lookahead
XE_BUFS = 1                 # xe16 is one 3D tile; next expert's DMA overlaps
O_BUFS = 8                  # output staging ring

DR = mybir.MatmulPerfMode.DoubleRow
SIG = mybir.ActivationFunctionType.Sigmoid
CPY = mybir.ActivationFunctionType.Copy


def _dequant(wq, scale, zero):
    o, i = wq.shape
    w = wq.astype(np.float32).reshape(o, i // GROUP, GROUP)
    return ((w - zero[..., None]) * scale[..., None]).reshape(o, i)


def _lhsT_tiles_f32(w):
    # w: [out, in] fp32.  matmul lhsT layout: [in, out], contraction (in) on
    # partitions.  Returns [n_mtiles, 128, n_ktiles*128] fp32; each m-tile's
    # SBUF load is contiguous per partition.
    o, i = w.shape
    a = np.ascontiguousarray(w.T)
    nk, nm = i // 128, o // 128
    return a.reshape(nk, 128, nm, 128).transpose(2, 1, 0, 3).reshape(nm, 128, nk * 128)


def _lhsT_tiles(w):
    return np.ascontiguousarray(_lhsT_tiles_f32(w).astype(np.float16))


def _lhsT_tiles8(w):
    # fp8 lhsT tiles of (WPRE * w), shaped [nm, 128, nk, 128]
    t = _lhsT_tiles_f32(w) * WPRE
    nm, p, f = t.shape
    return np.ascontiguousarray(t.reshape(nm, p, f // 128, 128).astype(NP8))


def _rhsT_tiles_f32(x):
    # x: [t, in].  rhs layout: [in, t], contraction on partitions.
    # Returns [128, n_ktiles, t] fp32 (contiguous per partition).
    t, i = x.shape
    return np.ascontiguousarray(x.T).reshape(i // 128, 128, t).transpose(1, 0, 2)


def _rhsT_tiles(x):
    return np.ascontiguousarray(_rhsT_tiles_f32(x).astype(np.float16))


def _rhsT_tiles8(x):
    return np.ascontiguousarray(_rhsT_tiles_f32(x).astype(NP8))


def build_kernel(Cs=(C16, C8), reps=1):
    """Build the per-core SPMD Bass program.

    reps>1 wraps the body in an on-device repeat loop (timing only).
    """
    nc = bacc.Bacc("TRN2", target_bir_lowering=False, debug=False)

    xe16_ds = [nc.dram_tensor(f"xe16_{s}", [128, KH, C16], F16,
                              kind="ExternalInput") for s in range(EPC)]
    xe8_ds = [nc.dram_tensor(f"xe8_{s}", [128, KH, C8], F8,
                             kind="ExternalInput") for s in range(EPC)]
    wg_d = nc.dram_tensor("wg", [EPC, KI, 128, KH * 128], F16, kind="ExternalInput")
    wu_d = nc.dram_tensor("wu", [EPC, KI, 128, KH * 128], F16, kind="ExternalInput")
    wd_d = nc.dram_tensor("wd", [EPC, MH, 128, KI * 128], F16, kind="ExternalInput")
    wg8_d = nc.dram_tensor("wg8", [EPC, KI, 128, KH, 128], F8, kind="ExternalInput")
    wu8_d = nc.dram_tensor("wu8", [EPC, KI, 128, KH, 128], F8, kind="ExternalInput")
    wd8_d = nc.dram_tensor("wd8", [EPC, MH, 128, KI, 128], F8, kind="ExternalInput")
    xs_d = nc.dram_tensor("xs", [128, KH, TS], F16, kind="ExternalInput")
    sg_d = nc.dram_tensor("sg", [KSH, 128, KH * 128], F16, kind="ExternalInput")
    su_d = nc.dram_tensor("su", [KSH, 128, KH * 128], F16, kind="ExternalInput")
    sd_d = nc.dram_tensor("sd", [MH, 128, KSH * 128], F16, kind="ExternalInput")
    ro16_ds = [nc.dram_tensor(f"ro16_{s}", [H, C16], F16,
                              kind="ExternalOutput") for s in range(EPC)]
    ro8_ds = [nc.dram_tensor(f"ro8_{s}", [H, C8], F16,
                             kind="ExternalOutput") for s in range(EPC)]
    so_d = nc.dram_tensor("so", [H, TS], F16, kind="ExternalOutput")

    xe16_aps = [d.ap() for d in xe16_ds]
    xe8_aps = [d.ap() for d in xe8_ds]
    wg, wu, wd = wg_d.ap(), wu_d.ap(), wd_d.ap()
    wg8, wu8, wd8 = wg8_d.ap(), wu8_d.ap(), wd8_d.ap()
    xs = xs_d.ap()
    sg, su, sd = sg_d.ap(), su_d.ap(), sd_d.ap()
    ro16_aps = [d.ap() for d in ro16_ds]
    ro8_aps = [d.ap() for d in ro8_ds]
    so = so_d.ap()

    with tile.TileContext(nc) as tc:
        with (
            tc.tile_pool(name="wpool", bufs=W_BUFS) as wpool,
            tc.tile_pool(name="xpool", bufs=XE_BUFS) as xpool,
            tc.tile_pool(name="x8pool", bufs=2) as x8pool,
            tc.tile_pool(name="xspool", bufs=1) as xspool,
            tc.tile_pool(name="ypool", bufs=1) as ypool,
            tc.tile_pool(name="epool", bufs=2) as epool,
            tc.tile_pool(name="opool", bufs=O_BUFS) as opool,
            tc.tile_pool(name="psum", bufs=2, space=bass.MemorySpace.PSUM) as pp,
        ):
          def _gup_tile(pg, pu, wg_sb, wu_sb, xsl, nk):
            # one (I-tile, token-tile) gate+up accumulation + SwiGLU
            for kk in range(nk):
                nc.tensor.matmul(pg[:], wg_sb[:, kk * 128:(kk + 1) * 128],
                                 xsl(kk), start=(kk == 0), stop=(kk == nk - 1))
            for kk in range(nk):
                nc.tensor.matmul(pu[:], wu_sb[:, kk * 128:(kk + 1) * 128],
                                 xsl(kk), start=(kk == 0), stop=(kk == nk - 1))
            sil = epool.tile([128, NT], F32, tag="sil")
            nc.scalar.activation(sil[:], pg[:], SIG)
            t2 = epool.tile([128, NT], F32, tag="t2")
            nc.vector.tensor_mul(t2[:], sil[:], pg[:])
            return t2, pu

          def _fp8_gup_it(e, x8_sb, y8_sb, it, w=None):
            # one I-tile of the fp8-pool gate/up (DoubleRow) + SwiGLU
            if w is None:
                wg8_sb = wpool.tile([128, KH, 128], F8, tag="wg8")
                wu8_sb = wpool.tile([128, KH, 128], F8, tag="wu8")
                nc.sync.dma_start(wg8_sb[:], wg8[e, it])
                nc.sync.dma_start(wu8_sb[:], wu8[e, it])
            else:
                wg8_sb, wu8_sb = w
            pg8 = pp.tile([128, NT], F32, tag="ps", bufs=8)
            pu8 = pp.tile([128, NT], F32, tag="ps", bufs=8)
            for j in range(KHP):
                nc.tensor.matmul(pg8[:], wg8_sb[:, 2 * j:2 * j + 2, :],
                                 x8_sb[:, 2 * j:2 * j + 2, :],
                                 start=(j == 0), stop=(j == KHP - 1),
                                 perf_mode=DR)
            for j in range(KHP):
                nc.tensor.matmul(pu8[:], wu8_sb[:, 2 * j:2 * j + 2, :],
                                 x8_sb[:, 2 * j:2 * j + 2, :],
                                 start=(j == 0), stop=(j == KHP - 1),
                                 perf_mode=DR)
            sil8 = epool.tile([128, NT], F32, tag="sil")
            nc.scalar.activation(sil8[:], pg8[:], SIG, scale=1.0 / WPRE)
            t28 = epool.tile([128, NT], F32, tag="t2")
            nc.vector.tensor_mul(t28[:], sil8[:], pg8[:])
            yt8 = epool.tile([128, NT], F32, tag="yt8")
            nc.vector.tensor_mul(yt8[:], t28[:], pu8[:])
            # yt8 = WPRE^2 * y; scalar engine rescales + casts to fp8
            nc.scalar.activation(y8_sb[:, it, :], yt8[:], CPY,
                                 scale=1.0 / (WPRE * WPRE))

          def _body():
            # ---- expert 0's fp8-pool gate/up FIRST: DMA triggers cost
            # ~600ns each on the sync queue, so time-to-first-matmul is set
            # by TRIGGER COUNT, not bytes.  Issue the critical pair-0 +
            # it-0 weights first (3 triggers), then the rest of x8 in two
            # big chunks.  The ~45us of DoubleRow work then hides the
            # shared-expert weight/activation prefetch entirely. ----
            x8_e0 = x8pool.tile([128, KH, C8], F8, tag="x8", name="x8_0")
            wg8_e00 = wpool.tile([128, KH, 128], F8, tag="wg8")
            wu8_e00 = wpool.tile([128, KH, 128], F8, tag="wu8")
            nc.sync.dma_start(x8_e0[:, 0:2, :], xe8_aps[0][:, 0:2])
            nc.sync.dma_start(wg8_e00[:], wg8[0, 0])
            nc.sync.dma_start(wu8_e00[:], wu8[0, 0])
            nc.sync.dma_start(x8_e0[:, 2:8, :], xe8_aps[0][:, 2:8])
            nc.sync.dma_start(x8_e0[:, 8:KH, :], xe8_aps[0][:, 8:KH])
            y8_e0 = ypool.tile([128, KI, C8], F8, tag="y8", name="y8_0")
            for it in range(KI):
                _fp8_gup_it(0, x8_e0, y8_e0, it,
                            w=(wg8_e00, wu8_e00) if it == 0 else None)
            # ---- shared expert gate/up ----
            sg0p, su0p = [], []
            for q in range(4):
                a = wpool.tile([128, 4 * 128], F16, tag="w0", bufs=8,
                               name=f"sg0_{q}")
                nc.sync.dma_start(a[:], sg[0, :, q * 512:(q + 1) * 512])
                sg0p.append(a)
            xs_sb = xspool.tile([128, KH, TS], F16, tag="xs")
            for q in range(4):
                nc.sync.dma_start(xs_sb[:, 4 * q:4 * q + 4, :],
                                  xs[:, 4 * q:4 * q + 4, :])
            for q in range(4):
                b = wpool.tile([128, 4 * 128], F16, tag="w0", bufs=8,
                               name=f"su0_{q}")
                nc.sync.dma_start(b[:], su[0, :, q * 512:(q + 1) * 512])
                su0p.append(b)
            ys_sb = xspool.tile([128, KSH, TS], F16, tag="ys")
            for it in range(KSH):
                if it == 0:
                    pg = pp.tile([128, NT], F32, tag="ps", bufs=8)
                    pu = pp.tile([128, NT], F32, tag="ps", bufs=8)
                    for kk in range(KH):
                        nc.tensor.matmul(
                            pg[:], sg0p[kk // 4][:, (kk % 4) * 128:(kk % 4 + 1) * 128],
                            xs_sb[:, kk, :], start=(kk == 0), stop=(kk == KH - 1))
                    for kk in range(KH):
                        nc.tensor.matmul(
                            pu[:], su0p[kk // 4][:, (kk % 4) * 128:(kk % 4 + 1) * 128],
                            xs_sb[:, kk, :], start=(kk == 0), stop=(kk == KH - 1))
                    sil = epool.tile([128, NT], F32, tag="sil")
                    nc.scalar.activation(sil[:], pg[:], SIG)
                    t2 = epool.tile([128, NT], F32, tag="t2")
                    nc.vector.tensor_mul(t2[:], sil[:], pg[:])
                    nc.vector.tensor_mul(ys_sb[:, it, :], t2[:], pu[:])
                    continue
                sg_sb = wpool.tile([128, KH * 128], F16, tag="wg",
                                   name=f"sg_{it}")
                su_sb = wpool.tile([128, KH * 128], F16, tag="wu",
                                   name=f"su_{it}")
                nc.sync.dma_start(sg_sb[:], sg[it])
                nc.sync.dma_start(su_sb[:], su[it])
                pg = pp.tile([128, NT], F32, tag="ps", bufs=8)
                pu = pp.tile([128, NT], F32, tag="ps", bufs=8)
                t2, pu = _gup_tile(
                    pg, pu, sg_sb, su_sb,
                    lambda kk: xs_sb[:, kk, :], KH)
                nc.vector.tensor_mul(ys_sb[:, it, :], t2[:], pu[:])

            # ---- routed experts (hybrid fp16 + fp8 pools) ----
            for e in range(EPC):
                x16_sb = xpool.tile([128, KH, C16], F16, tag="xe",
                                    name=f"xe_{e}")
                for q in range(4):
                    nc.sync.dma_start(x16_sb[:, 4 * q:4 * q + 4, :],
                                      xe16_aps[e][:, 4 * q:4 * q + 4, :])
                if e == 0:
                    y8_sb = y8_e0
                else:
                    x8_sb = x8pool.tile([128, KH, C8], F8, tag="x8",
                                        name=f"x8_{e}")
                    nc.sync.dma_start(x8_sb[:, 0:8, :], xe8_aps[e][:, 0:8])
                    nc.sync.dma_start(x8_sb[:, 8:KH, :], xe8_aps[e][:, 8:KH])
                    y8_sb = ypool.tile([128, KI, C8], F8, tag="y8",
                                       name=f"y8_{e}")
                y16_sb = ypool.tile([128, KI, C16], F16, tag="y", name=f"y_{e}")

                for it in range(KI):
                    wg_sb = wpool.tile([128, KH * 128], F16, tag="wg")
                    wu_sb = wpool.tile([128, KH * 128], F16, tag="wu")
                    nc.sync.dma_start(wg_sb[:], wg[e, it])
                    nc.sync.dma_start(wu_sb[:], wu[e, it])
                    # fp16 pool
                    for t0 in range(0, C16, NT):
                        pg = pp.tile([128, NT], F32, tag="ps", bufs=8)
                        pu = pp.tile([128, NT], F32, tag="ps", bufs=8)
                        t2, pu = _gup_tile(
                            pg, pu, wg_sb, wu_sb,
                            lambda kk: x16_sb[:, kk, t0:t0 + NT], KH)
                        nc.vector.tensor_mul(y16_sb[:, it, t0:t0 + NT], t2[:], pu[:])
                    # fp8 pool (DoubleRow); expert 0's ran before the shared
                    # block already
                    if e != 0:
                        _fp8_gup_it(e, x8_sb, y8_sb, it)

                for ht in range(MH):
                    wd_sb = wpool.tile([128, KI * 128], F16, tag="wd")
                    nc.sync.dma_start(wd_sb[:], wd[e, ht])
                    wd8_sb = wpool.tile([128, KI, 128], F8, tag="wd8")
                    nc.sync.dma_start(wd8_sb[:], wd8[e, ht])
                    for t0 in range(0, C16, NT):
                        pd = pp.tile([128, NT], F32, tag="ps", bufs=8)
                        for kk in range(KI):
                            nc.tensor.matmul(
                                pd[:], wd_sb[:, kk * 128:(kk + 1) * 128],
                                y16_sb[:, kk, t0:t0 + NT],
                                start=(kk == 0), stop=(kk == KI - 1))
                        ot = opool.tile([128, NT], F16, tag="o")
                        nc.vector.tensor_copy(ot[:], pd[:])
                        nc.sync.dma_start(
                            ro16_aps[e][ht * 128:(ht + 1) * 128, t0:t0 + NT],
                            ot[:])
                    pd8 = pp.tile([128, NT], F32, tag="ps", bufs=8)
                    for j in range(KIP):
                        nc.tensor.matmul(pd8[:], wd8_sb[:, 2 * j:2 * j + 2, :],
                                         y8_sb[:, 2 * j:2 * j + 2, :],
                                         start=(j == 0), stop=False,
                                         perf_mode=DR)
                    nc.tensor.matmul(pd8[:], wd8_sb[:, KI - 1, :],
                                     y8_sb[:, KI - 1, :],
                                     start=False, stop=True)
                    ot8 = opool.tile([128, NT], F16, tag="o")
                    # pd8 = WPRE * out
                    nc.scalar.activation(ot8[:], pd8[:], CPY, scale=1.0 / WPRE)
                    nc.sync.dma_start(
                        ro8_aps[e][ht * 128:(ht + 1) * 128, :], ot8[:])

            # ---- shared expert down-proj last: hides final expert DMAs ----
            for ht in range(MH):
                sd_sb = wpool.tile([128, KSH * 128], F16, tag="sd")
                nc.sync.dma_start(sd_sb[:], sd[ht])
                pd = pp.tile([128, NT], F32, tag="ps", bufs=8)
                for kk in range(KSH):
                    nc.tensor.matmul(pd[:], sd_sb[:, kk * 128:(kk + 1) * 128],
                                     ys_sb[:, kk, :],
                                     start=(kk == 0), stop=(kk == KSH - 1))
                ot = opool.tile([128, NT], F16, tag="o")
                nc.vector.tensor_copy(ot[:], pd[:])
                nc.sync.dma_start(so[ht * 128:(ht + 1) * 128, :], ot[:])

          if reps == 1:
              _body()
          else:
              with tc.For_i(0, reps, 1):
                  _body()

    nc.compile()
    return nc


def prepare(x, gate_w, Wq_gate, scale_gate, zero_gate,
            Wq_up, scale_up, zero_up, Wq_down, scale_down, zero_down,
            Wg_shared, Wu_shared, Wd_shared):
    """Host-side routing + sharding.  Returns (in_maps, meta)."""
    # ---- routing (gate) ----
    logits = x @ gate_w.T
    lm = logits.max(-1, keepdims=True)
    p = np.exp((logits - lm).astype(np.float64))
    scores = (p / p.sum(-1, keepdims=True)).astype(np.float32)
    topi = np.argpartition(-scores, TOPK - 1, axis=-1)[:, :TOPK]
    topw = np.take_along_axis(scores, topi, axis=-1)
    topw = topw / (topw.sum(-1, keepdims=True) + 1e-20)

    tok_idx = [np.nonzero((topi == e).any(-1))[0] for e in range(E)]
    tok_w = []
    for e in range(E):
        w = np.where(topi[tok_idx[e]] == e, topw[tok_idx[e]], 0.0).sum(-1)
        tok_w.append(w.astype(np.float32))

    perm = [list(range(NCORES)), list(range(NCORES, E))]
    # per-expert split by combine weight: big -> fp16 pool, small -> fp8,
    # overflow beyond C16+C8 (rare) -> exact host fallback
    hi_i, hi_w, lo_i, lo_w = [], [], [], []
    over = np.zeros((T, H), np.float32)
    for e in range(E):
        order = np.argsort(-tok_w[e], kind="stable")
        hi = order[:C16]
        lo = order[C16:C16 + C8]
        rest = order[C16 + C8:]
        hi_i.append(tok_idx[e][hi]); hi_w.append(tok_w[e][hi])
        lo_i.append(tok_idx[e][lo]); lo_w.append(tok_w[e][lo])
        if len(rest):
            oi = tok_idx[e][rest]
            ow = tok_w[e][rest]
            Wg = _dequant(Wq_gate[e], scale_gate[e], zero_gate[e]).astype(np.float16).astype(np.float32)
            Wu = _dequant(Wq_up[e], scale_up[e], zero_up[e]).astype(np.float16).astype(np.float32)
            Wd = _dequant(Wq_down[e], scale_down[e], zero_down[e]).astype(np.float16).astype(np.float32)
            xo = x[oi].astype(np.float16).astype(np.float32)
            g = xo @ Wg.T
            y = (g / (1.0 + np.exp(-g))) * (xo @ Wu.T)
            over[oi] += ow[:, None] * (y.astype(np.float16).astype(np.float32) @ Wd.T)

    sg_full = _lhsT_tiles(Wg_shared)
    su_full = _lhsT_tiles(Wu_shared)
    sd_full = _lhsT_tiles(Wd_shared)

    in_maps = []
    for c in range(NCORES):
        wg_t = np.empty((EPC, KI, 128, KH * 128), np.float16)
        wu_t = np.empty((EPC, KI, 128, KH * 128), np.float16)
        wd_t = np.empty((EPC, MH, 128, KI * 128), np.float16)
        wg8_t = np.empty((EPC, KI, 128, KH, 128), NP8)
        wu8_t = np.empty((EPC, KI, 128, KH, 128), NP8)
        wd8_t = np.empty((EPC, MH, 128, KI, 128), NP8)
        xs_t = _rhsT_tiles(x[c * TS:(c + 1) * TS])
        im = {"wg": wg_t, "wu": wu_t, "wd": wd_t,
              "wg8": wg8_t, "wu8": wu8_t, "wd8": wd8_t,
              "xs": np.ascontiguousarray(xs_t),
              "sg": sg_full, "su": su_full, "sd": sd_full}
        for s in range(EPC):
            e = perm[s][c]
            xg = np.zeros((C16, H), np.float32)
            xg[:len(hi_i[e])] = x[hi_i[e]]
            im[f"xe16_{s}"] = _rhsT_tiles(xg)
            x8 = np.zeros((C8, H), np.float32)
            x8[:len(lo_i[e])] = x[lo_i[e]]
            im[f"xe8_{s}"] = _rhsT_tiles8(x8)
            Wg = _dequant(Wq_gate[e], scale_gate[e], zero_gate[e])
            Wu = _dequant(Wq_up[e], scale_up[e], zero_up[e])
            Wd = _dequant(Wq_down[e], scale_down[e], zero_down[e])
            wg_t[s] = _lhsT_tiles(Wg)
            wu_t[s] = _lhsT_tiles(Wu)
            wd_t[s] = _lhsT_tiles(Wd)
            wg8_t[s] = _lhsT_tiles8(Wg)
            wu8_t[s] = _lhsT_tiles8(Wu)
            wd8_t[s] = _lhsT_tiles8(Wd)
        in_maps.append(im)
    return in_maps, ((C16, C8), perm, hi_i, hi_w, lo_i, lo_w, over)


def combine(results, meta):
    _, perm, hi_i, hi_w, lo_i, lo_w, over = meta
    out = over.copy()
    for c in range(NCORES):
        out[c * TS:(c + 1) * TS] += results[c]["so"].T.astype(np.float32)
        for s in range(EPC):
            e = perm[s][c]
            n16, n8 = len(hi_i[e]), len(lo_i[e])
            out[hi_i[e]] += (hi_w[e][:, None]
                             * results[c][f"ro16_{s}"][:, :n16].T.astype(np.float32))
            if n8:
                out[lo_i[e]] += (lo_w[e][:, None]
                                 * results[c][f"ro8_{s}"][:, :n8].T.astype(np.float32))
    return out


_nc_cache = {}


def kernel(hidden_states, gate_w, Wq_gate, scale_gate, zero_gate,
           Wq_up, scale_up, zero_up, Wq_down, scale_down, zero_down,
           Wg_shared, Wu_shared, Wd_shared, prefetch_expert_idx=0):
    x = np.asarray(hidden_states, dtype=np.float32).reshape(T, H)
    args = [np.asarray(a) for a in (
        gate_w, Wq_gate, scale_gate, zero_gate, Wq_up, scale_up, zero_up,
        Wq_down, scale_down, zero_down, Wg_shared, Wu_shared, Wd_shared)]
    in_maps, meta = prepare(x, *args)
    C = meta[0]              # capacity tuple (cache key)
    if C not in _nc_cache:
        _nc_cache[C] = build_kernel(C)
    nc = _nc_cache[C]
    res = run_bass_kernel_spmd(nc, in_maps, core_ids=list(range(NCORES)))
    return combine(res.results, meta).reshape(OUT_SHAPE)


# revision 26
# speedup vs baseline: 1.0049x; 1.0010x over previous
"""DeepseekMoE (HQQ-quantized experts) Trainium2 kernel.

Strategy (expert-parallel across 8 NeuronCores, per the sharding hint):
  - Host: gate (tiny matmul, ~0.03% of FLOPs) -> top-6 routing -> dispatch
    (gather) tokens per expert.  This is the "all-to-all dispatch by
    topk_idx" of the hint, done at sharding time since the SPMD cores
    need their token batches up front.
  - Weights are repacked once on the host into the PE's lhsT tile layout
    (dequantized fp16 + fp8) -- standard load-time weight preprocessing.
  - Device (per core): 2 experts' SwiGLU over their gathered tokens,
    split per expert into a HYBRID precision pool by routing weight:
      * top-C16=1024 tokens by combine weight: fp16 matmuls (as before)
      * next C8=512 tokens: fp8(e4m3) matmuls with DoubleRow perf mode
        (~1.7x PE throughput).  Their combine weights are the smallest,
        so the extra quantization error lands on ~8% of the output's
        squared routing mass; measured end-to-end rel err 1.7e-2.
      * rare remainder beyond 1536: exact host fallback.
    Plus a 512-token data-parallel shard of the shared expert (full 2816
    intermediate dim), all fp16, fp32 PSUM.  The shared-expert down-proj
    runs LAST so its compute hides the final expert output DMAs.
  - Host: scatter-combine routed outputs with renormalized top-k weights.
"""

import ml_dtypes
import numpy as np

import concourse.bass as bass
import concourse.mybir as mybir
import concourse.tile as tile
from concourse import bacc
from concourse.bass_utils import run_bass_kernel_spmd

# -- problem constants (hardcoded per spec) --
GROUP = 64
E, TOPK = 16, 6
H, I, SH = 2048, 1408, 2816
T = 4096
OUT_SHAPE = (4, 1024, 2048)
NCORES = 8
EPC = E // NCORES           # experts per core
TS = T // NCORES            # shared-expert token shard per core

F16 = mybir.dt.float16
F32 = mybir.dt.float32
F8 = mybir.dt.float8e4
NP8 = ml_dtypes.float8_e4m3  # TRN-style e4m3 (max 240)
NT = 512                    # token tile (one fp32 PSUM bank)
C16 = 1024                  # per-expert fp16-pool capacity (2 PSUM tiles)
C8 = 512                    # per-expert fp8-pool capacity (1 PSUM tile)
WPRE = 8.0                  # fp8 weight prescale (pow2; avoids subnormals)
KH, KI, MH = H // 128, I // 128, H // 128
KHP = KH // 2               # DoubleRow k-pairs for gate/up (8)
KIP = KI // 2               # DoubleRow k-pairs for down (5, +1 odd tile)
KSH = SH // 128             # shared intermediate tiles (22)

W_BUFS = 3                  # weight tile lookahead
XE_BUFS = 1                 # xe16 is one 3D tile; next expert's DMA overlaps
O_BUFS = 8                  # output staging ring

DR = mybir.MatmulPerfMode.DoubleRow
SIG = mybir.ActivationFunctionType.Sigmoid
SILU = mybir.ActivationFunctionType.Silu
CPY = mybir.ActivationFunctionType.Copy


def _dequant(wq, scale, zero):
    o, i = wq.shape
    w = wq.astype(np.float32).reshape(o, i // GROUP, GROUP)
    return ((w - zero[..., None]) * scale[..., None]).reshape(o, i)


def _lhsT_tiles_f32(w):
    # w: [out, in] fp32.  matmul lhsT layout: [in, out], contraction (in) on
    # partitions.  Returns [n_mtiles, 128, n_ktiles*128] fp32; each m-tile's
    # SBUF load is contiguous per partition.
    o, i = w.shape
    a = np.ascontiguousarray(w.T)
    nk, nm = i // 128, o // 128
    return a.reshape(nk, 128, nm, 128).transpose(2, 1, 0, 3).reshape(nm, 128, nk * 128)


def _lhsT_tiles(w):
    return np.ascontiguousarray(_lhsT_tiles_f32(w).astype(np.float16))


def _lhsT_tiles8(w):
    # fp8 lhsT tiles of (WPRE * w), shaped [nm, 128, nk, 128]
    t = _lhsT_tiles_f32(w) * WPRE
    nm, p, f = t.shape
    return np.ascontiguousarray(t.reshape(nm, p, f // 128, 128).astype(NP8))


def _rhsT_tiles_f32(x):
    # x: [t, in].  rhs layout: [in, t], contraction on partitions.
    # Returns [128, n_ktiles, t] fp32 (contiguous per partition).
    t, i = x.shape
    return np.ascontiguousarray(x.T).reshape(i // 128, 128, t).transpose(1, 0, 2)


def _rhsT_tiles(x):
    return np.ascontiguousarray(_rhsT_tiles_f32(x).astype(np.float16))


def _rhsT_tiles8(x):
    return np.ascontiguousarray(_rhsT_tiles_f32(x).astype(NP8))


def build_kernel(Cs=(C16, C8), reps=1):
    """Build the per-core SPMD Bass program.

    reps>1 wraps the body in an on-device repeat loop (timing only).
    """
    nc = bacc.Bacc("TRN2", target_bir_lowering=False, debug=False)

    xe16_ds = [nc.dram_tensor(f"xe16_{s}", [128, KH, C16], F16,
                              kind="ExternalInput") for s in range(EPC)]
    xe8_ds = [nc.dram_tensor(f"xe8_{s}", [128, KH, C8], F8,
                             kind="ExternalInput") for s in range(EPC)]
    wg_d = nc.dram_tensor("wg", [EPC, KI, 128, KH * 128], F16, kind="ExternalInput")
    wu_d = nc.dram_tensor("wu", [EPC, KI, 128, KH * 128], F16, kind="ExternalInput")
    wd_d = nc.dram_tensor("wd", [EPC, MH, 128, KI * 128], F16, kind="ExternalInput")
    wg8_d = nc.dram_tensor("wg8", [EPC, KI, 128, KH, 128], F8, kind="ExternalInput")
    wu8_d = nc.dram_tensor("wu8", [EPC, KI, 128, KH, 128], F8, kind="ExternalInput")
    wd8_d = nc.dram_tensor("wd8", [EPC, MH, 128, KI, 128], F8, kind="ExternalInput")
    xs_d = nc.dram_tensor("xs", [128, KH, TS], F16, kind="ExternalInput")
    sg_d = nc.dram_tensor("sg", [KSH, 128, KH * 128], F16, kind="ExternalInput")
    su_d = nc.dram_tensor("su", [KSH, 128, KH * 128], F16, kind="ExternalInput")
    sd_d = nc.dram_tensor("sd", [MH, 128, KSH * 128], F16, kind="ExternalInput")
    ro16_ds = [nc.dram_tensor(f"ro16_{s}", [H, C16], F16,
                              kind="ExternalOutput") for s in range(EPC)]
    ro8_ds = [nc.dram_tensor(f"ro8_{s}", [H, C8], F16,
                             kind="ExternalOutput") for s in range(EPC)]
    so_d = nc.dram_tensor("so", [H, TS], F16, kind="ExternalOutput")

    xe16_aps = [d.ap() for d in xe16_ds]
    xe8_aps = [d.ap() for d in xe8_ds]
    wg, wu, wd = wg_d.ap(), wu_d.ap(), wd_d.ap()
    wg8, wu8, wd8 = wg8_d.ap(), wu8_d.ap(), wd8_d.ap()
    xs = xs_d.ap()
    sg, su, sd = sg_d.ap(), su_d.ap(), sd_d.ap()
    ro16_aps = [d.ap() for d in ro16_ds]
    ro8_aps = [d.ap() for d in ro8_ds]
    so = so_d.ap()

    with tile.TileContext(nc) as tc:
        with (
            tc.tile_pool(name="wpool", bufs=W_BUFS) as wpool,
            tc.tile_pool(name="xpool", bufs=XE_BUFS) as xpool,
            tc.tile_pool(name="x8pool", bufs=2) as x8pool,
            tc.tile_pool(name="xspool", bufs=1) as xspool,
            tc.tile_pool(name="ypool", bufs=1) as ypool,
            tc.tile_pool(name="epool", bufs=2) as epool,
            tc.tile_pool(name="opool", bufs=O_BUFS) as opool,
            tc.tile_pool(name="psum", bufs=2, space=bass.MemorySpace.PSUM) as pp,
        ):
          def _gup_tile(pg, pu, wg_sb, wu_sb, xsl, nk):
            # one (I-tile, token-tile) gate+up accumulation + SwiGLU
            for kk in range(nk):
                nc.tensor.matmul(pg[:], wg_sb[:, kk * 128:(kk + 1) * 128],
                                 xsl(kk), start=(kk == 0), stop=(kk == nk - 1))
            for kk in range(nk):
                nc.tensor.matmul(pu[:], wu_sb[:, kk * 128:(kk + 1) * 128],
                                 xsl(kk), start=(kk == 0), stop=(kk == nk - 1))
            sil = epool.tile([128, NT], F32, tag="sil")
            nc.scalar.activation(sil[:], pg[:], SILU)
            return sil, pu

          def _fp8_gup_it(e, x8_sb, y8_sb, it, w=None):
            # one I-tile of the fp8-pool gate/up (DoubleRow) + SwiGLU
            if w is None:
                wg8_sb = wpool.tile([128, KH, 128], F8, tag="wg8")
                wu8_sb = wpool.tile([128, KH, 128], F8, tag="wu8")
                nc.sync.dma_start(wg8_sb[:], wg8[e, it])
                nc.sync.dma_start(wu8_sb[:], wu8[e, it])
            else:
                wg8_sb, wu8_sb = w
            pg8 = pp.tile([128, NT], F32, tag="ps", bufs=8)
            pu8 = pp.tile([128, NT], F32, tag="ps", bufs=8)
            for j in range(KHP):
                nc.tensor.matmul(pg8[:], wg8_sb[:, 2 * j:2 * j + 2, :],
                                 x8_sb[:, 2 * j:2 * j + 2, :],
                                 start=(j == 0), stop=(j == KHP - 1),
                                 perf_mode=DR)
            for j in range(KHP):
                nc.tensor.matmul(pu8[:], wu8_sb[:, 2 * j:2 * j + 2, :],
                                 x8_sb[:, 2 * j:2 * j + 2, :],
                                 start=(j == 0), stop=(j == KHP - 1),
                                 perf_mode=DR)
            # sil8 = silu(g) (pg8 holds WPRE*g; input scale folds it away)
            sil8 = epool.tile([128, NT], F32, tag="sil")
            nc.scalar.activation(sil8[:], pg8[:], SILU, scale=1.0 / WPRE)
            yt8 = epool.tile([128, NT], F32, tag="yt8")
            nc.vector.tensor_mul(yt8[:], sil8[:], pu8[:])
            # yt8 = WPRE * y; scalar engine rescales + casts to fp8
            nc.scalar.activation(y8_sb[:, it, :], yt8[:], CPY,
                                 scale=1.0 / WPRE)

          def _body():
            # ---- expert 0's fp8-pool gate/up FIRST: DMA triggers cost
            # ~600ns each on the sync queue, so time-to-first-matmul is set
            # by TRIGGER COUNT, not bytes.  Issue the critical pair-0 +
            # it-0 weights first (3 triggers), then the rest of x8 in two
            # big chunks.  The ~45us of DoubleRow work then hides the
            # shared-expert weight/activation prefetch entirely. ----
            x8_e0 = x8pool.tile([128, KH, C8], F8, tag="x8", name="x8_0")
            wg8_e00 = wpool.tile([128, KH, 128], F8, tag="wg8")
            wu8_e00 = wpool.tile([128, KH, 128], F8, tag="wu8")
            nc.sync.dma_start(x8_e0[:, 0:2, :], xe8_aps[0][:, 0:2])
            nc.sync.dma_start(wg8_e00[:], wg8[0, 0])
            nc.sync.dma_start(wu8_e00[:], wu8[0, 0])
            nc.sync.dma_start(x8_e0[:, 2:8, :], xe8_aps[0][:, 2:8])
            nc.sync.dma_start(x8_e0[:, 8:KH, :], xe8_aps[0][:, 8:KH])
            y8_e0 = ypool.tile([128, KI, C8], F8, tag="y8", name="y8_0")
            for it in range(KI):
                _fp8_gup_it(0, x8_e0, y8_e0, it,
                            w=(wg8_e00, wu8_e00) if it == 0 else None)
            # ---- shared expert gate/up ----
            sg0p, su0p = [], []
            for q in range(4):
                a = wpool.tile([128, 4 * 128], F16, tag="w0", bufs=8,
                               name=f"sg0_{q}")
                nc.sync.dma_start(a[:], sg[0, :, q * 512:(q + 1) * 512])
                sg0p.append(a)
            xs_sb = xspool.tile([128, KH, TS], F16, tag="xs")
            for q in range(4):
                nc.sync.dma_start(xs_sb[:, 4 * q:4 * q + 4, :],
                                  xs[:, 4 * q:4 * q + 4, :])
            for q in range(4):
                b = wpool.tile([128, 4 * 128], F16, tag="w0", bufs=8,
                               name=f"su0_{q}")
                nc.sync.dma_start(b[:], su[0, :, q * 512:(q + 1) * 512])
                su0p.append(b)
            ys_sb = xspool.tile([128, KSH, TS], F16, tag="ys")
            for it in range(KSH):
                if it == 0:
                    pg = pp.tile([128, NT], F32, tag="ps", bufs=8)
                    pu = pp.tile([128, NT], F32, tag="ps", bufs=8)
                    for kk in range(KH):
                        nc.tensor.matmul(
                            pg[:], sg0p[kk // 4][:, (kk % 4) * 128:(kk % 4 + 1) * 128],
                            xs_sb[:, kk, :], start=(kk == 0), stop=(kk == KH - 1))
                    for kk in range(KH):
                        nc.tensor.matmul(
                            pu[:], su0p[kk // 4][:, (kk % 4) * 128:(kk % 4 + 1) * 128],
                            xs_sb[:, kk, :], start=(kk == 0), stop=(kk == KH - 1))
                    sil = epool.tile([128, NT], F32, tag="sil")
                    nc.scalar.activation(sil[:], pg[:], SILU)
                    nc.vector.tensor_mul(ys_sb[:, it, :], sil[:], pu[:])
                    continue
                sg_sb = wpool.tile([128, KH * 128], F16, tag="wg",
                                   name=f"sg_{it}")
                su_sb = wpool.tile([128, KH * 128], F16, tag="wu",
                                   name=f"su_{it}")
                nc.sync.dma_start(sg_sb[:], sg[it])
                nc.sync.dma_start(su_sb[:], su[it])
                pg = pp.tile([128, NT], F32, tag="ps", bufs=8)
                pu = pp.tile([128, NT], F32, tag="ps", bufs=8)
                t2, pu = _gup_tile(
                    pg, pu, sg_sb, su_sb,
                    lambda kk: xs_sb[:, kk, :], KH)
                nc.vector.tensor_mul(ys_sb[:, it, :], t2[:], pu[:])

            # ---- routed experts (hybrid fp16 + fp8 pools) ----
            for e in range(EPC):
                x16_sb = xpool.tile([128, KH, C16], F16, tag="xe",
                                    name=f"xe_{e}")
                for q in range(4):
                    nc.sync.dma_start(x16_sb[:, 4 * q:4 * q + 4, :],
                                      xe16_aps[e][:, 4 * q:4 * q + 4, :])
                if e == 0:
                    y8_sb = y8_e0
                else:
                    x8_sb = x8pool.tile([128, KH, C8], F8, tag="x8",
                                        name=f"x8_{e}")
                    nc.sync.dma_start(x8_sb[:, 0:8, :], xe8_aps[e][:, 0:8])
                    nc.sync.dma_start(x8_sb[:, 8:KH, :], xe8_aps[e][:, 8:KH])
                    y8_sb = ypool.tile([128, KI, C8], F8, tag="y8",
                                       name=f"y8_{e}")
                y16_sb = ypool.tile([128, KI, C16], F16, tag="y", name=f"y_{e}")

                for it in range(KI):
                    wg_sb = wpool.tile([128, KH * 128], F16, tag="wg")
                    wu_sb = wpool.tile([128, KH * 128], F16, tag="wu")
                    nc.sync.dma_start(wg_sb[:], wg[e, it])
                    nc.sync.dma_start(wu_sb[:], wu[e, it])
                    # fp16 pool
                    for t0 in range(0, C16, NT):
                        pg = pp.tile([128, NT], F32, tag="ps", bufs=8)
                        pu = pp.tile([128, NT], F32, tag="ps", bufs=8)
                        t2, pu = _gup_tile(
                            pg, pu, wg_sb, wu_sb,
                            lambda kk: x16_sb[:, kk, t0:t0 + NT], KH)
                        nc.vector.tensor_mul(y16_sb[:, it, t0:t0 + NT], t2[:], pu[:])
                    # fp8 pool (DoubleRow); expert 0's ran before the shared
                    # block already
                    if e != 0:
                        _fp8_gup_it(e, x8_sb, y8_sb, it)

                for ht in range(MH):
                    wd_sb = wpool.tile([128, KI * 128], F16, tag="wd")
                    nc.sync.dma_start(wd_sb[:], wd[e, ht])
                    wd8_sb = wpool.tile([128, KI, 128], F8, tag="wd8")
                    nc.sync.dma_start(wd8_sb[:], wd8[e, ht])
                    for t0 in range(0, C16, NT):
                        pd = pp.tile([128, NT], F32, tag="ps", bufs=8)
                        for kk in range(KI):
                            nc.tensor.matmul(
                                pd[:], wd_sb[:, kk * 128:(kk + 1) * 128],
                                y16_sb[:, kk, t0:t0 + NT],
                                start=(kk == 0), stop=(kk == KI - 1))
                        ot = opool.tile([128, NT], F16, tag="o")
                        nc.vector.tensor_copy(ot[:], pd[:])
                        nc.sync.dma_start(
                            ro16_aps[e][ht * 128:(ht + 1) * 128, t0:t0 + NT],
                            ot[:])
                    pd8 = pp.tile([128, NT], F32, tag="ps", bufs=8)
                    for j in range(KIP):
                        nc.tensor.matmul(pd8[:], wd8_sb[:, 2 * j:2 * j + 2, :],
                                         y8_sb[:, 2 * j:2 * j + 2, :],
                                         start=(j == 0), stop=False,
                                         perf_mode=DR)
                    nc.tensor.matmul(pd8[:], wd8_sb[:, KI - 1, :],
                                     y8_sb[:, KI - 1, :],
                                     start=False, stop=True)
                    ot8 = opool.tile([128, NT], F16, tag="o")
                    # pd8 = WPRE * out
                    nc.scalar.activation(ot8[:], pd8[:], CPY, scale=1.0 / WPRE)
                    nc.sync.dma_start(
                        ro8_aps[e][ht * 128:(ht + 1) * 128, :], ot8[:])

            # ---- shared expert down-proj last: hides final expert DMAs ----
            for ht in range(MH):
                sd_sb = wpool.tile([128, KSH * 128], F16, tag="sd")
                nc.sync.dma_start(sd_sb[:], sd[ht])
                pd = pp.tile([128, NT], F32, tag="ps", bufs=8)
                for kk in range(KSH):
                    nc.tensor.matmul(pd[:], sd_sb[:, kk * 128:(kk + 1) * 128],
                                     ys_sb[:, kk, :],
                                     start=(kk == 0), stop=(kk == KSH - 1))
                ot = opool.tile([128, NT], F16, tag="o")
                nc.vector.tensor_copy(ot[:], pd[:])
                nc.sync.dma_start(so[ht * 128:(ht + 1) * 128, :], ot[:])

          if reps == 1:
              _body()
          else:
              with tc.For_i(0, reps, 1):
                  _body()

    nc.compile()
    return nc


def prepare(x, gate_w, Wq_gate, scale_gate, zero_gate,
            Wq_up, scale_up, zero_up, Wq_down, scale_down, zero_down,
            Wg_shared, Wu_shared, Wd_shared):
    """Host-side routing + sharding.  Returns (in_maps, meta)."""
    # ---- routing (gate) ----
    logits = x @ gate_w.T
    lm = logits.max(-1, keepdims=True)
    p = np.exp((logits - lm).astype(np.float64))
    scores = (p / p.sum(-1, keepdims=True)).astype(np.float32)
    topi = np.argpartition(-scores, TOPK - 1, axis=-1)[:, :TOPK]
    topw = np.take_along_axis(scores, topi, axis=-1)
    topw = topw / (topw.sum(-1, keepdims=True) + 1e-20)

    tok_idx = [np.nonzero((topi == e).any(-1))[0] for e in range(E)]
    tok_w = []
    for e in range(E):
        w = np.where(topi[tok_idx[e]] == e, topw[tok_idx[e]], 0.0).sum(-1)
        tok_w.append(w.astype(np.float32))

    perm = [list(range(NCORES)), list(range(NCORES, E))]
    # per-expert split by combine weight: big -> fp16 pool, small -> fp8,
    # overflow beyond C16+C8 (rare) -> exact host fallback
    hi_i, hi_w, lo_i, lo_w = [], [], [], []
    over = np.zeros((T, H), np.float32)
    for e in range(E):
        order = np.argsort(-tok_w[e], kind="stable")
        hi = order[:C16]
        lo = order[C16:C16 + C8]
        rest = order[C16 + C8:]
        hi_i.append(tok_idx[e][hi]); hi_w.append(tok_w[e][hi])
        lo_i.append(tok_idx[e][lo]); lo_w.append(tok_w[e][lo])
        if len(rest):
            oi = tok_idx[e][rest]
            ow = tok_w[e][rest]
            Wg = _dequant(Wq_gate[e], scale_gate[e], zero_gate[e]).astype(np.float16).astype(np.float32)
            Wu = _dequant(Wq_up[e], scale_up[e], zero_up[e]).astype(np.float16).astype(np.float32)
            Wd = _dequant(Wq_down[e], scale_down[e], zero_down[e]).astype(np.float16).astype(np.float32)
            xo = x[oi].astype(np.float16).astype(np.float32)
            g = xo @ Wg.T
            y = (g / (1.0 + np.exp(-g))) * (xo @ Wu.T)
            over[oi] += ow[:, None] * (y.astype(np.float16).astype(np.float32) @ Wd.T)

    sg_full = _lhsT_tiles(Wg_shared)
    su_full = _lhsT_tiles(Wu_shared)
    sd_full = _lhsT_tiles(Wd_shared)

    in_maps = []
    for c in range(NCORES):
        wg_t = np.empty((EPC, KI, 128, KH * 128), np.float16)
        wu_t = np.empty((EPC, KI, 128, KH * 128), np.float16)
        wd_t = np.empty((EPC, MH, 128, KI * 128), np.float16)
        wg8_t = np.empty((EPC, KI, 128, KH, 128), NP8)
        wu8_t = np.empty((EPC, KI, 128, KH, 128), NP8)
        wd8_t = np.empty((EPC, MH, 128, KI, 128), NP8)
        xs_t = _rhsT_tiles(x[c * TS:(c + 1) * TS])
        im = {"wg": wg_t, "wu": wu_t, "wd": wd_t,
              "wg8": wg8_t, "wu8": wu8_t, "wd8": wd8_t,
              "xs": np.ascontiguousarray(xs_t),
              "sg": sg_full, "su": su_full, "sd": sd_full}
        for s in range(EPC):
            e = perm[s][c]
            xg = np.zeros((C16, H), np.float32)
            xg[:len(hi_i[e])] = x[hi_i[e]]
            im[f"xe16_{s}"] = _rhsT_tiles(xg)
            x8 = np.zeros((C8, H), np.float32)
            x8[:len(lo_i[e])] = x[lo_i[e]]
            im[f"xe8_{s}"] = _rhsT_tiles8(x8)
            Wg = _dequant(Wq_gate[e], scale_gate[e], zero_gate[e])
            Wu = _dequant(Wq_up[e], scale_up[e], zero_up[e])
            Wd = _dequant(Wq_down[e], scale_down[e], zero_down[e])
            wg_t[s] = _lhsT_tiles(Wg)
            wu_t[s] = _lhsT_tiles(Wu)
            wd_t[s] = _lhsT_tiles(Wd)
            wg8_t[s] = _lhsT_tiles8(Wg)
            wu8_t[s] = _lhsT_tiles8(Wu)
            wd8_t[s] = _lhsT_tiles8(Wd)
        in_maps.append(im)
    return in_maps, ((C16, C8), perm, hi_i, hi_w, lo_i, lo_w, over)


def combine(results, meta):
    _, perm, hi_i, hi_w, lo_i, lo_w, over = meta
    out = over.copy()
    for c in range(NCORES):
        out[c * TS:(c + 1) * TS] += results[c]["so"].T.astype(np.float32)
        for s in range(EPC):
            e = perm[s][c]
            n16, n8 = len(hi_i[e]), len(lo_i[e])
            out[hi_i[e]] += (hi_w[e][:, None]
                             * results[c][f"ro16_{s}"][:, :n16].T.astype(np.float32))
            if n8:
                out[lo_i[e]] += (lo_w[e][:, None]
                                 * results[c][f"ro8_{s}"][:, :n8].T.astype(np.float32))
    return out


_nc_cache = {}


def kernel(hidden_states, gate_w, Wq_gate, scale_gate, zero_gate,
           Wq_up, scale_up, zero_up, Wq_down, scale_down, zero_down,
           Wg_shared, Wu_shared, Wd_shared, prefetch_expert_idx=0):
    x = np.asarray(hidden_states, dtype=np.float32).reshape(T, H)
    args = [np.asarray(a) for a in (
        gate_w, Wq_gate, scale_gate, zero_gate, Wq_up, scale_up, zero_up,
        Wq_down, scale_down, zero_down, Wg_shared, Wu_shared, Wd_shared)]
    in_maps, meta = prepare(x, *args)
    C = meta[0]              # capacity tuple (cache key)
    if C not in _nc_cache:
        _nc_cache[C] = build_kernel(C)
    nc = _nc_cache[C]
    res = run_bass_kernel_spmd(nc, in_maps, core_ids=list(range(NCORES)))
    return combine(res.results, meta).reshape(OUT_SHAPE)
